# revision 34
# baseline (speedup 1.0000x reference)
"""Trainium2 Bass kernel for nn_EnhancedCardAwarePolicy.

Strategy: pure data-parallel across 8 NeuronCores (batch 16384 -> 2048/core).

Key algebraic simplifications (exactly value-preserving vs the reference):
  * The hand self-attention MHA is dead code: the cross-attention that
    consumes it has sequence length 1, so its softmax is identically 1 and
    its output is independent of the query.  hand_ctx reduces to
        (8 / max(hand_size,1)) * (enemy_emb @ he_wv @ he_wo + he_bv @ he_wo + he_bo)
  * Card encodings are pure functions of the card index 0..53 -> fold the
    embedding tables into one [54, 32] table, and fold that table through
    the downstream linear layers, so the enemy-card path becomes matmuls
    against a one-hot [54, B] matrix built on-device.
  * strat_ctx's second linear layer is folded into cx_w1.
  * cx_w3 is folded into the action scorer's first layer (as_w1[:HID]) and
    the action-type classifier's first layer, so the `ctx` activation is
    never materialized.
  * The per-action tables are folded into per-action bias vectors on host.
  * softmax+bonus is computed unnormalized: out = score + (expl@Bm)/(expl@1).

Device layout: feature-major [D, B] activations in fp16 (fp32 PSUM accum);
relu(ctx1 + per-action-bias) runs as DVE tensor_scalar ops from SBUF at 4x
packed rate; z2 action matmuls col-tile the PE array in concurrent pairs.
"""

import numpy as np
import ml_dtypes
from contextlib import ExitStack

BF16 = np.float16

B = 16384
NCORES = 8
BC = B // NCORES          # 2048 batch rows per core
NCH = 4                   # chunks per core
N = BC // NCH             # 512 batch columns per chunk
A = 30                    # real actions
AP_ = 32                  # padded actions
E = 32
HID = 128

_cache = {}


# ---------------------------------------------------------------------------
# host-side folding
# ---------------------------------------------------------------------------

def _card_table(val_emb, suit_emb, type_emb):
    """[54, 32] full card encoding table, matching _encode_cards."""
    c = np.arange(54)
    invalid = (c == 0) | (c == 53)
    v = np.where(invalid, 0, (c - 1) % 13 + 1)
    s = np.where(invalid, 0, (c - 1) // 13 + 1)
    ce = np.concatenate([val_emb[v], suit_emb[s]], axis=-1)          # [54, 32]
    ct = np.where(v == 11, 1, np.where(v == 12, 2, np.where(v == 13, 3, 0)))
    te = type_emb[ct]                                                # [54, 8]
    pad = np.zeros((54, E - te.shape[-1]), np.float32)
    return (ce + np.concatenate([te, pad], axis=-1)).astype(np.float32)


def _action_fold(ac, card_emb, ce_w1, ce_b1, ce_w2, ce_b2,
                 as_w1, as_b1, as_b3):
    """Per-action biases + bonus matrix from action_card_indices [30, 4]."""
    ac = np.asarray(ac, np.int64)
    mask = ac != 0
    combo_size = mask.sum(1).astype(np.float32)
    values = np.where(mask, (ac - 1) % 13 + 1, 0)
    has_valid = mask.any(1)
    fidx = np.argmax(mask, axis=1)
    fv = values[np.arange(ac.shape[0]), fidx]
    same = np.where(mask, values == fv[:, None], True).all(1).astype(np.float32)
    vf = values.astype(np.float32)
    attack = np.where(values == 1, 1.0,
             np.where(values == 11, 10.0,
             np.where(values == 12, 15.0,
             np.where(values == 13, 20.0, vf))))
    total = (attack * mask).sum(1).astype(np.float32)
    suits = np.where(mask, (ac - 1) // 13 + 1, 0)
    uniq = sum((suits == s).any(1) for s in (1, 2, 3, 4)).astype(np.float32)
    ace = ((values == 1) & mask).any(1).astype(np.float32)
    valid = ((combo_size <= 4.0) & ((same > 0) | (ace > 0))).astype(np.float32)
    feats = np.stack([combo_size, same, total, uniq, ace, valid], 1)
    feats = np.where(has_valid[:, None], feats, 0.0).astype(np.float32)

    emb = card_emb[ac]                                   # [30, 4, 32]
    m = mask.astype(np.float32)[..., None]
    cnt = np.maximum(m.sum(1), 1.0)
    act_emb = (emb * m).sum(1) / cnt
    act_emb = np.where(has_valid[:, None], act_emb, 0.0).astype(np.float32)
    combo_enc = np.maximum(feats @ ce_w1 + ce_b1, 0.0) @ ce_w2 + ce_b2

    action_bias = act_emb @ as_w1[HID:HID + E] + combo_enc @ as_w1[HID + E:] + as_b1

    strength = feats[:, 2] / 20.0
    b3 = float(as_b3[0])
    Bm1 = np.zeros((4, AP_ + 2), np.float32)
    for a in range(A):
        if has_valid[a]:
            col = np.array([strength[a], 1.0 - strength[a], 0.0, 0.0])
        else:
            col = np.array([0.0, 0.0, 0.0, 2.0])
        Bm1[:, a] = col + b3
    Bm1[:, AP_] = 1.0                                    # denominator column
    ab = np.zeros((AP_, 64), np.float32)
    ab[:A] = action_bias
    return ab, Bm1


def _prep(inputs):
    """Fold weights, build per-core input maps. Returns (in_maps, consts)."""
    f32 = lambda x: np.ascontiguousarray(np.asarray(x), dtype=np.float32)
    hc = np.asarray(inputs["hand_cards"])        # [B, 8] int
    ec = np.asarray(inputs["enemy_card"])        # [B]
    hs = np.asarray(inputs["hand_size"])         # [B]
    gs = f32(inputs["game_state"])               # [B, 10]
    dc = f32(inputs["discard_pile_cards"])       # [B, 54]

    card_emb = _card_table(f32(inputs["val_emb"]), f32(inputs["suit_emb"]),
                           f32(inputs["type_emb"]))
    card_emb1 = np.concatenate([card_emb, np.ones((54, 1), np.float32)], 1)

    he_wv, he_bv = f32(inputs["he_wv"]), f32(inputs["he_bv"])
    he_wo, he_bo = f32(inputs["he_wo"]), f32(inputs["he_bo"])
    Mc = np.concatenate([he_wv @ he_wo, (he_bv @ he_wo + he_bo)[None]], 0)  # [33,32]
    A0s = 8.0 * (card_emb1 @ Mc)                                   # [54, 32]

    cx_w1, cx_b1 = f32(inputs["cx_w1"]), f32(inputs["cx_b1"])
    W1h = np.ascontiguousarray(cx_w1[0:E])                         # [32, 128]
    A2 = card_emb @ cx_w1[E:2 * E]                                 # [54, 128]
    W1s = cx_w1[2 * E:2 * E + 32]                                  # [32, 128]
    W1d = np.ascontiguousarray(cx_w1[2 * E + 32:])                 # [54, 128]
    se_w1, se_b1 = f32(inputs["se_w1"]).copy(), f32(inputs["se_b1"])
    se_w2, se_b2 = f32(inputs["se_w2"]), f32(inputs["se_b2"])
    U = se_w2 @ W1s                                                # [64, 128]
    b1f = cx_b1 + se_b2 @ W1s                                      # [128]
    se_w1[19] /= 4.0          # device computes suit-diversity count 0..4

    cxw2, cxb2 = f32(inputs["cx_w2"]), f32(inputs["cx_b2"])
    cxw3, cxb3 = f32(inputs["cx_w3"]), f32(inputs["cx_b3"])
    atw1, atb1 = f32(inputs["atc_w1"]), f32(inputs["atc_b1"])

    as_w1, as_b1 = f32(inputs["as_w1"]), f32(inputs["as_b1"])
    as_w2, as_b2 = f32(inputs["as_w2"]), f32(inputs["as_b2"])
    as_w3, as_b3 = f32(inputs["as_w3"]), f32(inputs["as_b3"])
    ab, Bm1 = _action_fold(inputs["action_card_indices"], card_emb,
                           f32(inputs["ce_w1"]), f32(inputs["ce_b1"]),
                           f32(inputs["ce_w2"]), f32(inputs["ce_b2"]),
                           as_w1, as_b1, as_b3)
    W1c = as_w1[:HID]                                              # [128, 64]
    # fold cx_w3 through the action scorer & type classifier
    Wfold = cxw3 @ W1c                                             # [128, 64]
    Wfoldd = np.concatenate([Wfold, Wfold], 1)                     # [128, 128]
    bias64 = cxb3 @ W1c                                            # [64]
    ab = ab + bias64[None, :]                                      # [32, 64]
    atw1p = cxw3 @ atw1                                            # [128, 64]
    atb1p = atb1 + cxb3 @ atw1                                     # [64]

    abp = np.zeros((128, 16), np.float32)
    for p in range(16):
        abp[0:64, p] = ab[2 * p]
        abp[64:128, p] = ab[2 * p + 1]
    W2blk = np.zeros((128, 64), np.float32)
    W2blk[0:64, 0:32] = as_w2
    W2blk[64:128, 32:64] = as_w2
    b2q = np.tile(as_b2, 4).astype(np.float32)                     # [128]
    w3blk = np.zeros((128, 4), np.float32)
    for i in range(4):
        w3blk[32 * i:32 * i + 32, i] = as_w3[:, 0]

    # sew1 padded to sh_in layout: rows 0:10 strat-w, 10:32 zero, 32:42 gs-w
    sew1v2 = np.zeros((42, 64), np.float32)
    sew1v2[0:10] = se_w1[10:20]
    sew1v2[32:42] = se_w1[0:10]

    # ---- fp16 const blob: each const occupies [0:P, c0:c0+W]
    M1 = np.zeros((128, 128), np.float32)
    M1[0:64] = U
    M1[64:118] = A2
    M2 = np.zeros((128, 128), np.float32)
    M2[0:32] = W1h
    M2[64:118] = W1d
    f16consts = [
        ("ident", np.eye(128, dtype=np.float32), 0),
        ("sew1", sew1v2, 0), ("A0s", A0s, 64), ("M1", M1, 0), ("M2", M2, 0),
        ("cxw2", cxw2, 0), ("Wfoldd", Wfoldd, 0),
        ("atw1p", atw1p, 0), ("atw2", f32(inputs["atc_w2"]), 0),
        ("W2blk", W2blk, 0), ("w3blk", w3blk, 0), ("Bm1", Bm1, 0),
    ]
    cols16 = {}
    c0 = 0
    for name, arr, r0 in f16consts:
        cols16[name] = (r0, arr.shape[0], c0, arr.shape[1])
        c0 += arr.shape[1]
    blob16 = np.zeros((128, c0), BF16)
    for name, arr, r0 in f16consts:
        r0, p, s, w = cols16[name]
        blob16[r0:r0 + p, s:s + w] = arr.astype(BF16)

    # ---- fp32 bias blob (per-partition column vectors for ACT bias)
    f32consts = [("seb1", se_b1), ("b1f", b1f), ("cxb2", cxb2),
                 ("atb1p", atb1p), ("atb2", f32(inputs["atc_b2"])),
                 ("b2q", b2q), ("iota54", np.arange(64, dtype=np.float32)),
                 ("abp", abp)]
    cols32 = {}
    c0 = 0
    for name, arr in f32consts:
        a2 = np.asarray(arr, np.float32)
        if a2.ndim == 1:
            a2 = a2[:, None]
        cols32[name] = (a2.shape[0], c0, a2.shape[1])
        c0 += a2.shape[1]
    blob32 = np.zeros((128, c0), np.float32)
    for name, arr in f32consts:
        a2 = np.asarray(arr, np.float32)
        if a2.ndim == 1:
            a2 = a2[:, None]
        p, s, w = cols32[name]
        blob32[0:p, s:s + w] = a2

    consts = {"blob16": blob16, "blob32": blob32,
              "_cols16": cols16, "_cols32": cols32}

    in_maps = []
    for i in range(NCORES):
        sl = slice(i * BC, (i + 1) * BC)
        m = {"blob16": blob16, "blob32": blob32}
        inB = np.zeros((191, BC), BF16)
        inB[32:86] = dc[sl].T
        inB[117:127] = gs[sl].T
        inB[127:191] = ec[sl].astype(BF16)[None, :]
        m["inB"] = inB
        hci = hc[sl].astype(BF16)
        hcB = np.zeros((128, 144), BF16)
        hcB[:, 0:128] = hci.reshape(16, 128, 8).transpose(1, 0, 2).reshape(128, 128)
        hcB[:, 128:144] = hs[sl].astype(BF16).reshape(16, 128).T
        m["hcB"] = hcB
        in_maps.append(m)
    return in_maps, consts


# ---------------------------------------------------------------------------
# device program
# ---------------------------------------------------------------------------

def _build(consts, n_r2_sc=7, n_s1_sc=0):
    import concourse.bass as bass
    import concourse.tile as tile
    import concourse.mybir as mybir
    from concourse import bacc

    dt = mybir.dt.float32
    dth = mybir.dt.float16
    AF = mybir.ActivationFunctionType
    OP = mybir.AluOpType
    AX = mybir.AxisListType

    cols16 = consts["_cols16"]
    cols32 = consts["_cols32"]

    nc = bacc.Bacc("TRN2", target_bir_lowering=False, debug=False,
                   enable_asserts=False, num_devices=NCORES)

    din = {}
    din["blob16"] = nc.dram_tensor("blob16", list(consts["blob16"].shape), dth,
                                   kind="ExternalInput").ap()
    din["blob32"] = nc.dram_tensor("blob32", list(consts["blob32"].shape), dt,
                                   kind="ExternalInput").ap()
    din["inB"] = nc.dram_tensor("inB", [191, BC], dth, kind="ExternalInput").ap()
    din["hcB"] = nc.dram_tensor("hcB", [128, 144], dth,
                                kind="ExternalInput").ap()
    out_d = nc.dram_tensor("out", [BC, A], dt, kind="ExternalOutput").ap()
    # out rows b = 512*n + 128*s + p  ->  [n][p, s, a]
    out_r = out_d.rearrange("(n s p) a -> n p s a", n=NCH, s=4, p=128)

    with tile.TileContext(nc) as tc, ExitStack() as ctx:
        ctx.enter_context(nc.allow_low_precision(
            reason="fp16 holds small exact integers / 2e-2 tolerance"))
        cpool = ctx.enter_context(tc.tile_pool(name="consts", bufs=1))
        core = ctx.enter_context(tc.tile_pool(name="core", bufs=1))
        work = ctx.enter_context(tc.tile_pool(name="work", bufs=3))
        s1p = ctx.enter_context(tc.tile_pool(name="s1p", bufs=8))
        s2p = ctx.enter_context(tc.tile_pool(name="s2p", bufs=4))
        fout = ctx.enter_context(tc.tile_pool(name="fout", bufs=2))
        ps_f = ctx.enter_context(tc.tile_pool(name="ps_f", bufs=2, space="PSUM"))
        ps_c = ctx.enter_context(tc.tile_pool(name="ps_c", bufs=1, space="PSUM"))
        ps_z = ctx.enter_context(tc.tile_pool(name="ps_z", bufs=2, space="PSUM"))
        wps = ctx.enter_context(tc.tile_pool(name="wps", bufs=1, space="PSUM"))
        ps_o = ctx.enter_context(tc.tile_pool(name="ps_o", bufs=2, space="PSUM"))

        # ---- consolidated DMAs (few descriptors, spread across idle queues)
        hcB = core.tile([128, 144], dth, tag="hcB")
        nc.sync.dma_start(hcB[:], din["hcB"])
        b16 = cpool.tile(list(consts["blob16"].shape), dth, tag="b16")
        nc.sync.dma_start(b16[:], din["blob16"])
        b32 = cpool.tile(list(consts["blob32"].shape), dt, tag="b32")
        nc.sync.dma_start(b32[:], din["blob32"])

        # HAM warmup: ~5us of back-to-back dummy matmuls while waiting for
        # input DMAs + hand-feature block; flips the PE clock gate to 8/8
        # before real matmuls start (it then stays warm: no gap > 3.4us).
        wtile = wps.tile([128, N], dt, tag="warm")
        for wi in range(12):
            nc.tensor.matmul(wtile[:], b16[:, 0:128], b16[:, 0:512],
                             start=True, stop=True)

        def pe_filler(k=2):
            # dep-free matmuls: issue instantly when the PE would otherwise
            # stall, keeping the HAM activity window busy (clock stays 8/8)
            for _ in range(k):
                nc.tensor.matmul(wtile[:, 0:256], b16[:, 0:128],
                                 b16[:, 0:256], start=True, stop=True)

        def pf(k=2):
            for _ in range(k):
                nc.tensor.matmul(wtile[:, 0:128], b16[:, 0:128],
                                 b16[:, 0:128], start=True, stop=True)
        dcX = core.tile([128, BC], dth, tag="dcX")
        nc.scalar.dma_start(dcX[32:128, :], din["inB"][0:96, :])
        sh_in = core.tile([42, BC], dth, tag="sh_in")   # strat+r | zeros | gs
        nc.sync.dma_start(sh_in[11:42, :], din["inB"][96:127, :])
        ecbc = core.tile([64, BC], dth, tag="ecbc")
        nc.scalar.dma_start(ecbc[:], din["inB"][127:191, :])

        def c16(name):
            r0, p, s, w = cols16[name]
            return b16[r0:r0 + p, s:s + w]

        def c32(name):
            p, s, w = cols32[name]
            return b32[0:p, s:s + w]

        hcS = hcB[:, 0:128]
        S = core.tile([128, 176], dth, tag="S")          # 11 blocks of 16
        expl = core.tile([4, BC], dth, tag="expl")

        g = nc.vector
        v = nc.vector
        sc = nc.scalar

        iota_col = c32("iota54")
        g.tensor_copy(S[:, 0:16], hcB[:, 128:144])       # hand_size batch-major

        # ---- per-card features (DVE, batch-major fp16, exact integer ops)
        ft = {k: core.tile([128, 128], dth, tag=f"ft_{k}", name=f"ft_{k}") for k in
              ("t", "g13", "g26", "g39", "s0", "m13", "v0", "mask",
               "s0p", "ace", "face", "lowd", "low", "su1", "su2", "su3", "su4")}
        g.tensor_scalar(ft["t"][:], hcS, -1.0, None, OP.add)
        g.tensor_scalar(ft["g13"][:], ft["t"][:], 13.0, None, OP.is_ge)
        g.tensor_scalar(ft["g26"][:], ft["t"][:], 26.0, None, OP.is_ge)
        g.tensor_scalar(ft["g39"][:], ft["t"][:], 39.0, None, OP.is_ge)
        g.tensor_tensor(ft["s0"][:], ft["g13"][:], ft["g26"][:], OP.add)
        g.tensor_tensor(ft["s0"][:], ft["s0"][:], ft["g39"][:], OP.add)
        g.tensor_scalar(ft["m13"][:], ft["s0"][:], 13.0, None, OP.mult)
        g.tensor_tensor(ft["v0"][:], ft["t"][:], ft["m13"][:], OP.subtract)
        g.tensor_scalar(ft["mask"][:], hcS, 0.5, None, OP.is_ge)
        g.tensor_scalar(ft["s0p"][:], ft["s0"][:], 1.0, None, OP.add)
        g.tensor_tensor(ft["s0p"][:], ft["s0p"][:], ft["mask"][:], OP.mult)
        g.tensor_scalar(ft["ace"][:], ft["v0"][:], 0.0, None, OP.is_equal)
        g.tensor_scalar(ft["face"][:], ft["v0"][:], 10.0, None, OP.is_ge)
        g.tensor_scalar(ft["lowd"][:], ft["v0"][:], 1.0, None, OP.is_ge)
        g.tensor_scalar(ft["low"][:], ft["v0"][:], 5.0, None, OP.is_le)
        g.tensor_tensor(ft["low"][:], ft["low"][:], ft["lowd"][:], OP.mult)
        for k, s in (("su1", 1.0), ("su2", 2.0), ("su3", 3.0), ("su4", 4.0)):
            g.tensor_scalar(ft[k][:], ft["s0p"][:], s, None, OP.is_equal)

        # ---- reduce 8 cards -> per-batch sums into S blocks (DVE)
        for blk, k in ((1, "ace"), (2, "face"), (3, "low"),
                       (4, "su1"), (5, "su2"), (6, "su3"), (7, "su4")):
            src = ft[k].rearrange("p (j c) -> p j c", c=8)
            v.tensor_reduce(S[:, 16 * blk:16 * blk + 16], src, AX.X, OP.add)

        # ---- hvr, sdiv, r (batch-major small tiles)
        hsr = core.tile([128, 16], dt, tag="hsr")
        v.tensor_scalar(hsr[:], S[:, 0:16], 1e-8, None, OP.add)
        v.reciprocal(hsr[:], hsr[:])
        v.tensor_tensor(S[:, 128:144], S[:, 32:48], hsr[:], OP.mult)  # hvr
        ge = [core.tile([128, 16], dth, tag=f"ge{k}", name=f"ge{k}") for k in range(4)]
        for k in range(4):
            v.tensor_scalar(ge[k][:], S[:, 64 + 16 * k:80 + 16 * k], 0.5, None,
                            OP.is_ge)
        v.tensor_tensor(ge[0][:], ge[0][:], ge[1][:], OP.add)
        v.tensor_tensor(ge[2][:], ge[2][:], ge[3][:], OP.add)
        v.tensor_tensor(S[:, 144:160], ge[0][:], ge[2][:], OP.add)   # sdiv cnt
        rmax = core.tile([128, 16], dt, tag="rmax")
        v.tensor_scalar(rmax[:], S[:, 0:16], 1.0, None, OP.max)
        rr32 = core.tile([128, 16], dt, tag="rr32")
        v.reciprocal(rr32[:], rmax[:])
        v.tensor_copy(S[:, 160:176], rr32[:])                        # r (fp16)


        def front_gen(n, st):
            cols = slice(N * n, N * (n + 1))

            # -- rotate per-batch scalars into rows: 4 transposes of [128, 11]
            scalT = ps_f.tile([128, N], dth, tag="fe", name="scalT")
            S_kj = S.rearrange("p (k j) -> p j k", j=16)
            for s in range(4):
                nc.tensor.transpose(scalT[0:11, 128 * s:128 * (s + 1)],
                                    S_kj[:, 4 * n + s, :], c16("ident"))
            sc.activation(sh_in[0:11, cols], scalT[0:11, :], AF.Copy)
            pf(2)
            yield

            # -- X1: rows 0:64 strat-hidden relu, 64:128 enemy one-hot
            X1 = work.tile([128, N], dth, tag="X1")
            v.tensor_scalar(X1[64:128, :], ecbc[:, cols], iota_col, None,
                            OP.is_equal)
            shp = ps_f.tile([128, N], dt, tag="fe")
            nc.tensor.matmul(shp[0:64, :], c16("sew1"), sh_in[0:42, cols],
                             start=True, stop=True)
            sc.activation(X1[0:64, :], shp[0:64, :], AF.Relu, bias=c32("seb1"))
            pf(2)
            yield

            # -- hand_ctx = (A0s^T oh) * r -> dcX rows 0:32
            rrow = work.tile([1, N], dth, tag="rrow")
            nc.sync.dma_start(rrow[:], sh_in[10:11, cols])
            r32 = work.tile([32, N], dth, tag="r32")
            nc.gpsimd.partition_broadcast(r32[:], rrow[:], channels=32)
            yps = ps_f.tile([128, N], dt, tag="fe")
            nc.tensor.matmul(yps[0:32, :], c16("A0s"), X1[64:118, :],
                             start=True, stop=True)
            v.tensor_tensor(dcX[0:32, cols], yps[0:32, :], r32[:], OP.mult)
            pf(2)
            yield

            # -- z1 = M1^T X1 + M2^T dcX + b1f   (two full-K matmuls)
            z1 = ps_f.tile([128, N], dt, tag="fe")
            nc.tensor.matmul(z1[:], c16("M1"), X1[:], start=True, stop=False)
            nc.tensor.matmul(z1[:], c16("M2"), dcX[:, cols], start=False,
                             stop=True)
            h1 = work.tile([128, N], dth, tag="h1")
            sc.activation(h1[:], z1[:], AF.Relu, bias=c32("b1f"))
            pf(2)
            yield

            h2p = ps_f.tile([128, N], dt, tag="fe")
            nc.tensor.matmul(h2p[:], c16("cxw2"), h1[:], start=True, stop=True)
            h2 = work.tile([128, N], dth, tag="h2")
            sc.activation(h2[:], h2p[:], AF.Relu, bias=c32("cxb2"))
            st["h2"] = h2
            pf(2)
            yield

            # -- action-type probs (unnormalized exp), cx_w3 folded in
            tphp = ps_f.tile([128, N], dt, tag="fe")
            nc.tensor.matmul(tphp[0:64, :], c16("atw1p"), h2[:],
                             start=True, stop=True)
            tph = work.tile([64, N], dth, tag="tph")
            sc.activation(tph[:], tphp[0:64, :], AF.Relu, bias=c32("atb1p"))
            pf(2)
            yield

            tlp = ps_f.tile([128, N], dt, tag="fe")
            nc.tensor.matmul(tlp[0:4, :], c16("atw2"), tph[:],
                             start=True, stop=True)
            sc.activation(expl[:, cols], tlp[0:4, :], AF.Exp, bias=c32("atb2"))
            pf(2)
            yield

            # -- action MLP input: ctx1 duplicated pair (cx_w3 folded)
            ctx1d = ps_c.tile([128, N], dt, tag="ctx1d")
            nc.tensor.matmul(ctx1d[:], c16("Wfoldd"), h2[:], start=True,
                             stop=True)
            c1d = work.tile([128, N], dth, tag="c1d")
            sc.activation(c1d[:], ctx1d[:], AF.Copy)
            pf(2)
            st["c1d"] = c1d

        def action_gen(n, st):
            c1d = st["c1d"]
            score = ps_o.tile([128, 128], dt, tag="fin")
            abp = c32("abp")
            for q in range(8):
                s1pair = []
                for p in (2 * q, 2 * q + 1):
                    t1 = s1p.tile([128, N], dth, tag="s1",
                                  name=f"s1_{n}_{p}", bufs=6)
                    v.tensor_scalar(t1[:], c1d[:], abp[:, p:p + 1], 0.0,
                                    OP.add, OP.max)
                    s1pair.append(t1)
                z2q = ps_z.tile([128, N], dt, tag="z2", name=f"z2_{n}_{q}")
                nc.tensor.matmul(z2q[0:64, :], c16("W2blk"), s1pair[0][:],
                                 start=True, stop=True)
                nc.tensor.matmul(z2q[64:128, :], c16("W2blk"), s1pair[1][:],
                                 start=True, stop=True)
                on_sc = q < n_r2_sc
                t = s2p.tile([128, N], dth, tag="s2a" if on_sc else "s2v",
                             name=f"s2_{n}_{q}", bufs=3)
                if on_sc:
                    sc.activation(t[:], z2q[:], AF.Relu, bias=c32("b2q"))
                else:
                    v.tensor_scalar(t[:], z2q[:], c32("b2q"), 0.0,
                                    OP.add, OP.max)
                for s in range(4):
                    nc.tensor.matmul(score[:, 32 * s + 4 * q:32 * s + 4 * q + 4],
                                     t[:, 128 * s:128 * (s + 1)],
                                     c16("w3blk"), start=True, stop=True)
                pe_filler(1)
                yield

            numer = ps_o.tile([128, 136], dt, tag="fin")
            for s in range(4):
                nc.tensor.matmul(numer[:, 34 * s:34 * (s + 1)],
                                 expl[:, N * n + 128 * s:N * n + 128 * (s + 1)],
                                 c16("Bm1"), start=True, stop=True)
            yield

            recipT = fout.tile([128, 4], dt, tag="recip")
            den = numer.rearrange("p (s c) -> p s c", c=34)[:, :, 32]
            v.reciprocal(recipT[:], den)
            tmp = fout.tile([128, 120], dt, tag="tmp")
            for s in range(4):
                v.tensor_scalar(tmp[:, 30 * s:30 * (s + 1)],
                                numer[:, 34 * s:34 * s + 30],
                                recipT[:, s:s + 1], None, OP.mult)
            sc_ap = score.rearrange("p (s c) -> p s c", c=32)[:, :, 0:30]
            outT = fout.tile([128, 120], dt, tag="outT")
            v.tensor_tensor(outT.rearrange("p (s c) -> p s c", c=30),
                            tmp.rearrange("p (s c) -> p s c", c=30),
                            sc_ap, OP.add)
            nc.sync.dma_start(out_r[n],
                              outT.rearrange("p (s c) -> p s c", c=30))

        def drain(g):
            for _ in g:
                pass

        # software pipeline: chunk m's action stage interleaves with
        # chunk (m+2)'s front stage so the in-order PE always has work
        sts = [dict() for _ in range(NCH)]
        fgens = [front_gen(n, sts[n]) for n in range(NCH)]
        agens = [action_gen(n, sts[n]) for n in range(NCH)]
        drain(fgens[0])
        drain(fgens[1])
        for m in range(NCH):
            f = fgens[m + 2] if m + 2 < NCH else None
            a = agens[m]
            alive = True
            while alive:
                alive = False
                try:
                    next(a)
                    alive = True
                except StopIteration:
                    pass
                if f is not None:
                    try:
                        next(f)
                        alive = True
                    except StopIteration:
                        pass

    nc.compile()
    return nc


def _get_program(consts):
    key = "prog"
    if key not in _cache:
        _cache[key] = _build(consts)
    return _cache[key]


def kernel(**inputs):
    in_maps, consts = _prep(inputs)
    nc = _get_program(consts)
    from concourse.bass_utils import run_bass_kernel_spmd
    res = run_bass_kernel_spmd(nc, in_maps, core_ids=list(range(NCORES)))
    out = np.concatenate([res.results[i]["out"] for i in range(NCORES)], 0)
    return out.astype(np.float32)


# revision 35
# speedup vs baseline: 1.2135x; 1.2135x over previous
"""Trainium2 Bass kernel for nn_EnhancedCardAwarePolicy.

Strategy: pure data-parallel across 8 NeuronCores (batch 16384 -> 2048/core).

Key algebraic simplifications (exactly value-preserving vs the reference):
  * The hand self-attention MHA is dead code: the cross-attention that
    consumes it has sequence length 1, so its softmax is identically 1 and
    its output is independent of the query.  hand_ctx reduces to
        (8 / max(hand_size,1)) * (enemy_emb @ he_wv @ he_wo + he_bv @ he_wo + he_bo)
  * Card encodings are pure functions of the card index 0..53 -> fold the
    embedding tables into one [54, 32] table, and fold that table through
    the downstream linear layers, so the enemy-card path becomes matmuls
    against a one-hot [54, B] matrix built on-device.
  * strat_ctx's second linear layer is folded into cx_w1.
  * cx_w3 is folded into the action scorer's first layer (as_w1[:HID]) and
    the action-type classifier's first layer, so the `ctx` activation is
    never materialized.
  * The per-action tables are folded into per-action bias vectors on host.
  * softmax+bonus is computed unnormalized: out = score + (expl@Bm)/(expl@1).

Device layout: feature-major [D, B] activations in fp16 (fp32 PSUM accum);
relu(ctx1 + per-action-bias) runs as DVE tensor_scalar ops from SBUF at 4x
packed rate; z2 action matmuls col-tile the PE array in concurrent pairs.
"""

import numpy as np
import ml_dtypes
from contextlib import ExitStack

BF16 = np.float16

B = 16384
NCORES = 8
BC = B // NCORES          # 2048 batch rows per core
NCH = 4                   # chunks per core
N = BC // NCH             # 512 batch columns per chunk
A = 30                    # real actions
AP_ = 32                  # padded actions
E = 32
HID = 128

_cache = {}


# ---------------------------------------------------------------------------
# host-side folding
# ---------------------------------------------------------------------------

def _card_table(val_emb, suit_emb, type_emb):
    """[54, 32] full card encoding table, matching _encode_cards."""
    c = np.arange(54)
    invalid = (c == 0) | (c == 53)
    v = np.where(invalid, 0, (c - 1) % 13 + 1)
    s = np.where(invalid, 0, (c - 1) // 13 + 1)
    ce = np.concatenate([val_emb[v], suit_emb[s]], axis=-1)          # [54, 32]
    ct = np.where(v == 11, 1, np.where(v == 12, 2, np.where(v == 13, 3, 0)))
    te = type_emb[ct]                                                # [54, 8]
    pad = np.zeros((54, E - te.shape[-1]), np.float32)
    return (ce + np.concatenate([te, pad], axis=-1)).astype(np.float32)


def _action_fold(ac, card_emb, ce_w1, ce_b1, ce_w2, ce_b2,
                 as_w1, as_b1, as_b3):
    """Per-action biases + bonus matrix from action_card_indices [30, 4]."""
    ac = np.asarray(ac, np.int64)
    mask = ac != 0
    combo_size = mask.sum(1).astype(np.float32)
    values = np.where(mask, (ac - 1) % 13 + 1, 0)
    has_valid = mask.any(1)
    fidx = np.argmax(mask, axis=1)
    fv = values[np.arange(ac.shape[0]), fidx]
    same = np.where(mask, values == fv[:, None], True).all(1).astype(np.float32)
    vf = values.astype(np.float32)
    attack = np.where(values == 1, 1.0,
             np.where(values == 11, 10.0,
             np.where(values == 12, 15.0,
             np.where(values == 13, 20.0, vf))))
    total = (attack * mask).sum(1).astype(np.float32)
    suits = np.where(mask, (ac - 1) // 13 + 1, 0)
    uniq = sum((suits == s).any(1) for s in (1, 2, 3, 4)).astype(np.float32)
    ace = ((values == 1) & mask).any(1).astype(np.float32)
    valid = ((combo_size <= 4.0) & ((same > 0) | (ace > 0))).astype(np.float32)
    feats = np.stack([combo_size, same, total, uniq, ace, valid], 1)
    feats = np.where(has_valid[:, None], feats, 0.0).astype(np.float32)

    emb = card_emb[ac]                                   # [30, 4, 32]
    m = mask.astype(np.float32)[..., None]
    cnt = np.maximum(m.sum(1), 1.0)
    act_emb = (emb * m).sum(1) / cnt
    act_emb = np.where(has_valid[:, None], act_emb, 0.0).astype(np.float32)
    combo_enc = np.maximum(feats @ ce_w1 + ce_b1, 0.0) @ ce_w2 + ce_b2

    action_bias = act_emb @ as_w1[HID:HID + E] + combo_enc @ as_w1[HID + E:] + as_b1

    strength = feats[:, 2] / 20.0
    b3 = float(as_b3[0])
    Bm1 = np.zeros((4, AP_ + 2), np.float32)
    for a in range(A):
        if has_valid[a]:
            col = np.array([strength[a], 1.0 - strength[a], 0.0, 0.0])
        else:
            col = np.array([0.0, 0.0, 0.0, 2.0])
        Bm1[:, a] = col + b3
    Bm1[:, AP_] = 1.0                                    # denominator column
    ab = np.zeros((AP_, 64), np.float32)
    ab[:A] = action_bias
    return ab, Bm1


def _prep(inputs):
    """Fold weights, build per-core input maps. Returns (in_maps, consts)."""
    f32 = lambda x: np.ascontiguousarray(np.asarray(x), dtype=np.float32)
    hc = np.asarray(inputs["hand_cards"])        # [B, 8] int
    ec = np.asarray(inputs["enemy_card"])        # [B]
    hs = np.asarray(inputs["hand_size"])         # [B]
    gs = f32(inputs["game_state"])               # [B, 10]
    dc = f32(inputs["discard_pile_cards"])       # [B, 54]

    card_emb = _card_table(f32(inputs["val_emb"]), f32(inputs["suit_emb"]),
                           f32(inputs["type_emb"]))
    card_emb1 = np.concatenate([card_emb, np.ones((54, 1), np.float32)], 1)

    he_wv, he_bv = f32(inputs["he_wv"]), f32(inputs["he_bv"])
    he_wo, he_bo = f32(inputs["he_wo"]), f32(inputs["he_bo"])
    Mc = np.concatenate([he_wv @ he_wo, (he_bv @ he_wo + he_bo)[None]], 0)  # [33,32]
    A0s = 8.0 * (card_emb1 @ Mc)                                   # [54, 32]

    cx_w1, cx_b1 = f32(inputs["cx_w1"]), f32(inputs["cx_b1"])
    W1h = np.ascontiguousarray(cx_w1[0:E])                         # [32, 128]
    A2 = card_emb @ cx_w1[E:2 * E]                                 # [54, 128]
    W1s = cx_w1[2 * E:2 * E + 32]                                  # [32, 128]
    W1d = np.ascontiguousarray(cx_w1[2 * E + 32:])                 # [54, 128]
    se_w1, se_b1 = f32(inputs["se_w1"]).copy(), f32(inputs["se_b1"])
    se_w2, se_b2 = f32(inputs["se_w2"]), f32(inputs["se_b2"])
    U = se_w2 @ W1s                                                # [64, 128]
    b1f = cx_b1 + se_b2 @ W1s                                      # [128]
    se_w1[19] /= 4.0          # device computes suit-diversity count 0..4

    cxw2, cxb2 = f32(inputs["cx_w2"]), f32(inputs["cx_b2"])
    cxw3, cxb3 = f32(inputs["cx_w3"]), f32(inputs["cx_b3"])
    atw1, atb1 = f32(inputs["atc_w1"]), f32(inputs["atc_b1"])

    as_w1, as_b1 = f32(inputs["as_w1"]), f32(inputs["as_b1"])
    as_w2, as_b2 = f32(inputs["as_w2"]), f32(inputs["as_b2"])
    as_w3, as_b3 = f32(inputs["as_w3"]), f32(inputs["as_b3"])
    ab, Bm1 = _action_fold(inputs["action_card_indices"], card_emb,
                           f32(inputs["ce_w1"]), f32(inputs["ce_b1"]),
                           f32(inputs["ce_w2"]), f32(inputs["ce_b2"]),
                           as_w1, as_b1, as_b3)
    W1c = as_w1[:HID]                                              # [128, 64]
    # fold cx_w3 through the action scorer & type classifier
    Wfold = cxw3 @ W1c                                             # [128, 64]
    Wfoldd = np.concatenate([Wfold, Wfold], 1)                     # [128, 128]
    bias64 = cxb3 @ W1c                                            # [64]
    ab = ab + bias64[None, :]                                      # [32, 64]
    atw1p = cxw3 @ atw1                                            # [128, 64]
    atb1p = atb1 + cxb3 @ atw1                                     # [64]

    abp = np.zeros((128, 16), np.float32)
    for p in range(16):
        abp[0:64, p] = ab[2 * p]
        abp[64:128, p] = ab[2 * p + 1]
    W2blk = np.zeros((128, 64), np.float32)
    W2blk[0:64, 0:32] = as_w2
    W2blk[64:128, 32:64] = as_w2
    b2q = np.tile(as_b2, 4).astype(np.float32)                     # [128]
    w3blk = np.zeros((128, 4), np.float32)
    for i in range(4):
        w3blk[32 * i:32 * i + 32, i] = as_w3[:, 0]

    # sew1 padded to sh_in layout: rows 0:10 strat-w, 10:32 zero, 32:42 gs-w
    sew1v2 = np.zeros((42, 64), np.float32)
    sew1v2[0:10] = se_w1[10:20]
    sew1v2[32:42] = se_w1[0:10]

    # ---- fp16 const blob: each const occupies [0:P, c0:c0+W]
    M1 = np.zeros((128, 128), np.float32)
    M1[0:64] = U
    M1[64:118] = A2
    M2 = np.zeros((128, 128), np.float32)
    M2[0:32] = W1h
    M2[64:118] = W1d
    f16consts = [
        ("ident", np.eye(128, dtype=np.float32), 0),
        ("sew1", sew1v2, 0), ("A0s", A0s, 64), ("M1", M1, 0), ("M2", M2, 0),
        ("cxw2", cxw2, 0), ("Wfoldd", Wfoldd, 0),
        ("atw1p", atw1p, 0), ("atw2", f32(inputs["atc_w2"]), 0),
        ("W2blk", W2blk, 0), ("w3blk", w3blk, 0), ("Bm1", Bm1, 0),
    ]
    cols16 = {}
    c0 = 0
    for name, arr, r0 in f16consts:
        cols16[name] = (r0, arr.shape[0], c0, arr.shape[1])
        c0 += arr.shape[1]
    blob16 = np.zeros((128, c0), BF16)
    for name, arr, r0 in f16consts:
        r0, p, s, w = cols16[name]
        blob16[r0:r0 + p, s:s + w] = arr.astype(BF16)

    # ---- fp32 bias blob (per-partition column vectors for ACT bias)
    f32consts = [("seb1", se_b1), ("b1f", b1f), ("cxb2", cxb2),
                 ("atb1p", atb1p), ("atb2", f32(inputs["atc_b2"])),
                 ("b2q", b2q), ("iota54", np.arange(64, dtype=np.float32)),
                 ("abp", abp)]
    cols32 = {}
    c0 = 0
    for name, arr in f32consts:
        a2 = np.asarray(arr, np.float32)
        if a2.ndim == 1:
            a2 = a2[:, None]
        cols32[name] = (a2.shape[0], c0, a2.shape[1])
        c0 += a2.shape[1]
    blob32 = np.zeros((128, c0), np.float32)
    for name, arr in f32consts:
        a2 = np.asarray(arr, np.float32)
        if a2.ndim == 1:
            a2 = a2[:, None]
        p, s, w = cols32[name]
        blob32[0:p, s:s + w] = a2

    consts = {"blob16": blob16, "blob32": blob32,
              "_cols16": cols16, "_cols32": cols32}

    in_maps = []
    for i in range(NCORES):
        sl = slice(i * BC, (i + 1) * BC)
        m = {"blob16": blob16, "blob32": blob32}
        inB = np.zeros((191, BC), BF16)
        inB[32:86] = dc[sl].T
        inB[117:127] = gs[sl].T
        inB[127:191] = ec[sl].astype(BF16)[None, :]
        m["inB"] = inB
        hci = hc[sl].astype(BF16)
        hcB = np.zeros((128, 144), BF16)
        hcB[:, 0:128] = hci.reshape(16, 128, 8).transpose(1, 0, 2).reshape(128, 128)
        hcB[:, 128:144] = hs[sl].astype(BF16).reshape(16, 128).T
        m["hcB"] = hcB
        in_maps.append(m)
    return in_maps, consts


# ---------------------------------------------------------------------------
# device program
# ---------------------------------------------------------------------------

def _build(consts, n_r2_sc=8, n_s1_sc=0):
    import concourse.bass as bass
    import concourse.tile as tile
    import concourse.mybir as mybir
    from concourse import bacc

    dt = mybir.dt.float32
    dth = mybir.dt.float16
    AF = mybir.ActivationFunctionType
    OP = mybir.AluOpType
    AX = mybir.AxisListType

    cols16 = consts["_cols16"]
    cols32 = consts["_cols32"]

    nc = bacc.Bacc("TRN2", target_bir_lowering=False, debug=False,
                   enable_asserts=False, num_devices=NCORES)

    din = {}
    din["blob16"] = nc.dram_tensor("blob16", list(consts["blob16"].shape), dth,
                                   kind="ExternalInput").ap()
    din["blob32"] = nc.dram_tensor("blob32", list(consts["blob32"].shape), dt,
                                   kind="ExternalInput").ap()
    din["inB"] = nc.dram_tensor("inB", [191, BC], dth, kind="ExternalInput").ap()
    din["hcB"] = nc.dram_tensor("hcB", [128, 144], dth,
                                kind="ExternalInput").ap()
    out_d = nc.dram_tensor("out", [BC, A], dt, kind="ExternalOutput").ap()
    # out rows b = 512*n + 128*s + p  ->  [n][p, s, a]
    out_r = out_d.rearrange("(n s p) a -> n p s a", n=NCH, s=4, p=128)

    with tile.TileContext(nc) as tc, ExitStack() as ctx:
        ctx.enter_context(nc.allow_low_precision(
            reason="fp16 holds small exact integers / 2e-2 tolerance"))
        cpool = ctx.enter_context(tc.tile_pool(name="consts", bufs=1))
        core = ctx.enter_context(tc.tile_pool(name="core", bufs=1))
        work = ctx.enter_context(tc.tile_pool(name="work", bufs=3))
        s1p = ctx.enter_context(tc.tile_pool(name="s1p", bufs=8))
        s2p = ctx.enter_context(tc.tile_pool(name="s2p", bufs=4))
        fout = ctx.enter_context(tc.tile_pool(name="fout", bufs=2))
        ps_f = ctx.enter_context(tc.tile_pool(name="ps_f", bufs=2, space="PSUM"))
        ps_c = ctx.enter_context(tc.tile_pool(name="ps_c", bufs=1, space="PSUM"))
        ps_z = ctx.enter_context(tc.tile_pool(name="ps_z", bufs=2, space="PSUM"))
        wps = ctx.enter_context(tc.tile_pool(name="wps", bufs=1, space="PSUM"))
        ps_o = ctx.enter_context(tc.tile_pool(name="ps_o", bufs=2, space="PSUM"))

        # ---- consolidated DMAs (few descriptors, spread across idle queues)
        hcB = core.tile([128, 144], dth, tag="hcB")
        nc.sync.dma_start(hcB[:], din["hcB"])
        b16 = cpool.tile(list(consts["blob16"].shape), dth, tag="b16")
        nc.sync.dma_start(b16[:], din["blob16"])
        b32 = cpool.tile(list(consts["blob32"].shape), dt, tag="b32")
        nc.sync.dma_start(b32[:], din["blob32"])

        # HAM warmup: ~5us of back-to-back dummy matmuls while waiting for
        # input DMAs + hand-feature block; flips the PE clock gate to 8/8
        # before real matmuls start (it then stays warm: no gap > 3.4us).
        wtile = wps.tile([128, N], dt, tag="warm")
        for wi in range(12):
            nc.tensor.matmul(wtile[:], b16[:, 0:128], b16[:, 0:512],
                             start=True, stop=True)

        def pe_filler(k=2):
            # dep-free matmuls: issue instantly when the PE would otherwise
            # stall, keeping the HAM activity window busy (clock stays 8/8)
            for _ in range(k):
                nc.tensor.matmul(wtile[:, 0:256], b16[:, 0:128],
                                 b16[:, 0:256], start=True, stop=True)

        def pf(k=2):
            for _ in range(k):
                nc.tensor.matmul(wtile[:, 0:128], b16[:, 0:128],
                                 b16[:, 0:128], start=True, stop=True)
        dcX = core.tile([128, BC], dth, tag="dcX")
        nc.scalar.dma_start(dcX[32:128, :], din["inB"][0:96, :])
        sh_in = core.tile([42, BC], dth, tag="sh_in")   # strat+r | zeros | gs
        nc.sync.dma_start(sh_in[11:42, :], din["inB"][96:127, :])
        ecbc = core.tile([64, BC], dth, tag="ecbc")
        nc.scalar.dma_start(ecbc[:], din["inB"][127:191, :])

        def c16(name):
            r0, p, s, w = cols16[name]
            return b16[r0:r0 + p, s:s + w]

        def c32(name):
            p, s, w = cols32[name]
            return b32[0:p, s:s + w]

        hcS = hcB[:, 0:128]
        S = core.tile([128, 176], dth, tag="S")          # 11 blocks of 16
        expl = core.tile([4, BC], dth, tag="expl")

        g = nc.vector
        v = nc.vector
        sc = nc.scalar

        iota_col = c32("iota54")
        g.tensor_copy(S[:, 0:16], hcB[:, 128:144])       # hand_size batch-major

        # ---- per-card features (DVE, batch-major fp16, exact integer ops)
        ft = {k: core.tile([128, 128], dth, tag=f"ft_{k}", name=f"ft_{k}") for k in
              ("t", "g13", "g26", "g39", "s0", "m13", "v0", "mask",
               "s0p", "ace", "face", "lowd", "low", "su1", "su2", "su3", "su4")}
        g.tensor_scalar(ft["t"][:], hcS, -1.0, None, OP.add)
        g.tensor_scalar(ft["g13"][:], ft["t"][:], 13.0, None, OP.is_ge)
        g.tensor_scalar(ft["g26"][:], ft["t"][:], 26.0, None, OP.is_ge)
        g.tensor_scalar(ft["g39"][:], ft["t"][:], 39.0, None, OP.is_ge)
        g.tensor_tensor(ft["s0"][:], ft["g13"][:], ft["g26"][:], OP.add)
        g.tensor_tensor(ft["s0"][:], ft["s0"][:], ft["g39"][:], OP.add)
        g.tensor_scalar(ft["m13"][:], ft["s0"][:], 13.0, None, OP.mult)
        g.tensor_tensor(ft["v0"][:], ft["t"][:], ft["m13"][:], OP.subtract)
        g.tensor_scalar(ft["mask"][:], hcS, 0.5, None, OP.is_ge)
        g.tensor_scalar(ft["s0p"][:], ft["s0"][:], 1.0, None, OP.add)
        g.tensor_tensor(ft["s0p"][:], ft["s0p"][:], ft["mask"][:], OP.mult)
        g.tensor_scalar(ft["ace"][:], ft["v0"][:], 0.0, None, OP.is_equal)
        g.tensor_scalar(ft["face"][:], ft["v0"][:], 10.0, None, OP.is_ge)
        g.tensor_scalar(ft["lowd"][:], ft["v0"][:], 1.0, None, OP.is_ge)
        g.tensor_scalar(ft["low"][:], ft["v0"][:], 5.0, None, OP.is_le)
        g.tensor_tensor(ft["low"][:], ft["low"][:], ft["lowd"][:], OP.mult)
        for k, s in (("su1", 1.0), ("su2", 2.0), ("su3", 3.0), ("su4", 4.0)):
            g.tensor_scalar(ft[k][:], ft["s0p"][:], s, None, OP.is_equal)

        # ---- reduce 8 cards -> per-batch sums into S blocks (DVE)
        for blk, k in ((1, "ace"), (2, "face"), (3, "low"),
                       (4, "su1"), (5, "su2"), (6, "su3"), (7, "su4")):
            src = ft[k].rearrange("p (j c) -> p j c", c=8)
            v.tensor_reduce(S[:, 16 * blk:16 * blk + 16], src, AX.X, OP.add)

        # ---- hvr, sdiv, r (batch-major small tiles)
        hsr = core.tile([128, 16], dt, tag="hsr")
        v.tensor_scalar(hsr[:], S[:, 0:16], 1e-8, None, OP.add)
        v.reciprocal(hsr[:], hsr[:])
        v.tensor_tensor(S[:, 128:144], S[:, 32:48], hsr[:], OP.mult)  # hvr
        ge = [core.tile([128, 16], dth, tag=f"ge{k}", name=f"ge{k}") for k in range(4)]
        for k in range(4):
            v.tensor_scalar(ge[k][:], S[:, 64 + 16 * k:80 + 16 * k], 0.5, None,
                            OP.is_ge)
        v.tensor_tensor(ge[0][:], ge[0][:], ge[1][:], OP.add)
        v.tensor_tensor(ge[2][:], ge[2][:], ge[3][:], OP.add)
        v.tensor_tensor(S[:, 144:160], ge[0][:], ge[2][:], OP.add)   # sdiv cnt
        rmax = core.tile([128, 16], dt, tag="rmax")
        v.tensor_scalar(rmax[:], S[:, 0:16], 1.0, None, OP.max)
        rr32 = core.tile([128, 16], dt, tag="rr32")
        v.reciprocal(rr32[:], rmax[:])
        v.tensor_copy(S[:, 160:176], rr32[:])                        # r (fp16)


        def front_gen(n, st):
            cols = slice(N * n, N * (n + 1))

            # -- rotate per-batch scalars into rows: 4 transposes of [128, 11]
            scalT = ps_f.tile([128, N], dth, tag="fe", name="scalT")
            S_kj = S.rearrange("p (k j) -> p j k", j=16)
            for s in range(4):
                nc.tensor.transpose(scalT[0:11, 128 * s:128 * (s + 1)],
                                    S_kj[:, 4 * n + s, :], c16("ident"))
            sc.activation(sh_in[0:11, cols], scalT[0:11, :], AF.Copy)

            yield

            # -- X1: rows 0:64 strat-hidden relu, 64:128 enemy one-hot
            X1 = work.tile([128, N], dth, tag="X1")
            v.tensor_scalar(X1[64:128, :], ecbc[:, cols], iota_col, None,
                            OP.is_equal)
            shp = ps_f.tile([128, N], dt, tag="fe")
            nc.tensor.matmul(shp[0:64, :], c16("sew1"), sh_in[0:42, cols],
                             start=True, stop=True)
            sc.activation(X1[0:64, :], shp[0:64, :], AF.Relu, bias=c32("seb1"))

            yield

            # -- hand_ctx = (A0s^T oh) * r -> dcX rows 0:32
            rrow = work.tile([1, N], dth, tag="rrow")
            nc.sync.dma_start(rrow[:], sh_in[10:11, cols])
            r32 = work.tile([32, N], dth, tag="r32")
            nc.gpsimd.partition_broadcast(r32[:], rrow[:], channels=32)
            yps = ps_f.tile([128, N], dt, tag="fe")
            nc.tensor.matmul(yps[0:32, :], c16("A0s"), X1[64:118, :],
                             start=True, stop=True)
            v.tensor_tensor(dcX[0:32, cols], yps[0:32, :], r32[:], OP.mult)

            yield

            # -- z1 = M1^T X1 + M2^T dcX + b1f   (two full-K matmuls)
            z1 = ps_f.tile([128, N], dt, tag="fe")
            nc.tensor.matmul(z1[:], c16("M1"), X1[:], start=True, stop=False)
            nc.tensor.matmul(z1[:], c16("M2"), dcX[:, cols], start=False,
                             stop=True)
            h1 = work.tile([128, N], dth, tag="h1")
            sc.activation(h1[:], z1[:], AF.Relu, bias=c32("b1f"))

            yield

            h2p = ps_f.tile([128, N], dt, tag="fe")
            nc.tensor.matmul(h2p[:], c16("cxw2"), h1[:], start=True, stop=True)
            h2 = work.tile([128, N], dth, tag="h2")
            sc.activation(h2[:], h2p[:], AF.Relu, bias=c32("cxb2"))
            st["h2"] = h2

            yield

            # -- action-type probs (unnormalized exp), cx_w3 folded in
            tphp = ps_f.tile([128, N], dt, tag="fe")
            nc.tensor.matmul(tphp[0:64, :], c16("atw1p"), h2[:],
                             start=True, stop=True)
            tph = work.tile([64, N], dth, tag="tph")
            sc.activation(tph[:], tphp[0:64, :], AF.Relu, bias=c32("atb1p"))

            yield

            tlp = ps_f.tile([128, N], dt, tag="fe")
            nc.tensor.matmul(tlp[0:4, :], c16("atw2"), tph[:],
                             start=True, stop=True)
            sc.activation(expl[:, cols], tlp[0:4, :], AF.Exp, bias=c32("atb2"))

            yield

            # -- action MLP input: ctx1 duplicated pair (cx_w3 folded)
            ctx1d = ps_c.tile([128, N], dt, tag="ctx1d")
            nc.tensor.matmul(ctx1d[:], c16("Wfoldd"), h2[:], start=True,
                             stop=True)
            c1d = work.tile([128, N], dth, tag="c1d")
            sc.activation(c1d[:], ctx1d[:], AF.Copy)
            st["c1d"] = c1d

        def action_gen(n, st):
            c1d = st["c1d"]
            score = ps_o.tile([128, 128], dt, tag="fin")
            abp = c32("abp")
            for q in range(8):
                s1pair = []
                for p in (2 * q, 2 * q + 1):
                    t1 = s1p.tile([128, N], dth, tag="s1",
                                  name=f"s1_{n}_{p}", bufs=6)
                    v.tensor_scalar(t1[:], c1d[:], abp[:, p:p + 1], 0.0,
                                    OP.add, OP.max)
                    s1pair.append(t1)
                z2q = ps_z.tile([128, N], dt, tag="z2", name=f"z2_{n}_{q}")
                nc.tensor.matmul(z2q[0:64, :], c16("W2blk"), s1pair[0][:],
                                 start=True, stop=True)
                nc.tensor.matmul(z2q[64:128, :], c16("W2blk"), s1pair[1][:],
                                 start=True, stop=True)
                on_sc = q < n_r2_sc
                t = s2p.tile([128, N], dth, tag="s2a" if on_sc else "s2v",
                             name=f"s2_{n}_{q}", bufs=3)
                if on_sc:
                    sc.activation(t[:], z2q[:], AF.Relu, bias=c32("b2q"))
                else:
                    v.tensor_scalar(t[:], z2q[:], c32("b2q"), 0.0,
                                    OP.add, OP.max)
                for s in range(4):
                    nc.tensor.matmul(score[:, 32 * s + 4 * q:32 * s + 4 * q + 4],
                                     t[:, 128 * s:128 * (s + 1)],
                                     c16("w3blk"), start=True, stop=True)
                yield

            numer = ps_o.tile([128, 136], dt, tag="fin")
            for s in range(4):
                nc.tensor.matmul(numer[:, 34 * s:34 * (s + 1)],
                                 expl[:, N * n + 128 * s:N * n + 128 * (s + 1)],
                                 c16("Bm1"), start=True, stop=True)
            yield

            recipT = fout.tile([128, 4], dt, tag="recip")
            den = numer.rearrange("p (s c) -> p s c", c=34)[:, :, 32]
            v.reciprocal(recipT[:], den)
            tmp = fout.tile([128, 120], dt, tag="tmp")
            for s in range(4):
                v.tensor_scalar(tmp[:, 30 * s:30 * (s + 1)],
                                numer[:, 34 * s:34 * s + 30],
                                recipT[:, s:s + 1], None, OP.mult)
            sc_ap = score.rearrange("p (s c) -> p s c", c=32)[:, :, 0:30]
            outT = fout.tile([128, 120], dt, tag="outT")
            v.tensor_tensor(outT.rearrange("p (s c) -> p s c", c=30),
                            tmp.rearrange("p (s c) -> p s c", c=30),
                            sc_ap, OP.add)
            nc.sync.dma_start(out_r[n],
                              outT.rearrange("p (s c) -> p s c", c=30))

        def drain(g):
            for _ in g:
                pass

        # software pipeline: chunk m's action stage interleaves with
        # chunk (m+2)'s front stage so the in-order PE always has work
        sts = [dict() for _ in range(NCH)]
        fgens = [front_gen(n, sts[n]) for n in range(NCH)]
        agens = [action_gen(n, sts[n]) for n in range(NCH)]
        drain(fgens[0])
        drain(fgens[1])
        for m in range(NCH):
            f = fgens[m + 2] if m + 2 < NCH else None
            a = agens[m]
            alive = True
            while alive:
                alive = False
                try:
                    next(a)
                    alive = True
                except StopIteration:
                    pass
                if f is not None:
                    try:
                        next(f)
                        alive = True
                    except StopIteration:
                        pass

    nc.compile()
    return nc


def _get_program(consts):
    key = "prog"
    if key not in _cache:
        _cache[key] = _build(consts)
    return _cache[key]


def kernel(**inputs):
    in_maps, consts = _prep(inputs)
    nc = _get_program(consts)
    from concourse.bass_utils import run_bass_kernel_spmd
    res = run_bass_kernel_spmd(nc, in_maps, core_ids=list(range(NCORES)))
    out = np.concatenate([res.results[i]["out"] for i in range(NCORES)], 0)
    return out.astype(np.float32)


# revision 36
# speedup vs baseline: 1.2499x; 1.0299x over previous
"""Trainium2 Bass kernel for nn_EnhancedCardAwarePolicy.

Strategy: pure data-parallel across 8 NeuronCores (batch 16384 -> 2048/core).

Key algebraic simplifications (exactly value-preserving vs the reference):
  * The hand self-attention MHA is dead code: the cross-attention that
    consumes it has sequence length 1, so its softmax is identically 1 and
    its output is independent of the query.  hand_ctx reduces to
        (8 / max(hand_size,1)) * (enemy_emb @ he_wv @ he_wo + he_bv @ he_wo + he_bo)
  * Card encodings are pure functions of the card index 0..53 -> fold the
    embedding tables into one [54, 32] table, and fold that table through
    the downstream linear layers, so the enemy-card path becomes matmuls
    against a one-hot [54, B] matrix built on-device.
  * strat_ctx's second linear layer is folded into cx_w1.
  * cx_w3 is folded into the action scorer's first layer (as_w1[:HID]) and
    the action-type classifier's first layer, so the `ctx` activation is
    never materialized.
  * The per-action tables are folded into per-action bias vectors on host.
  * softmax+bonus is computed unnormalized: out = score + (expl@Bm)/(expl@1).

Device layout: feature-major [D, B] activations in fp16 (fp32 PSUM accum);
relu(ctx1 + per-action-bias) runs as DVE tensor_scalar ops from SBUF at 4x
packed rate; z2 action matmuls col-tile the PE array in concurrent pairs.
"""

import numpy as np
import ml_dtypes
from contextlib import ExitStack

BF16 = np.float16

B = 16384
NCORES = 8
BC = B // NCORES          # 2048 batch rows per core
NCH = 4                   # chunks per core
N = BC // NCH             # 512 batch columns per chunk
A = 30                    # real actions
AP_ = 32                  # padded actions
E = 32
HID = 128

_cache = {}


# ---------------------------------------------------------------------------
# host-side folding
# ---------------------------------------------------------------------------

def _card_table(val_emb, suit_emb, type_emb):
    """[54, 32] full card encoding table, matching _encode_cards."""
    c = np.arange(54)
    invalid = (c == 0) | (c == 53)
    v = np.where(invalid, 0, (c - 1) % 13 + 1)
    s = np.where(invalid, 0, (c - 1) // 13 + 1)
    ce = np.concatenate([val_emb[v], suit_emb[s]], axis=-1)          # [54, 32]
    ct = np.where(v == 11, 1, np.where(v == 12, 2, np.where(v == 13, 3, 0)))
    te = type_emb[ct]                                                # [54, 8]
    pad = np.zeros((54, E - te.shape[-1]), np.float32)
    return (ce + np.concatenate([te, pad], axis=-1)).astype(np.float32)


def _action_fold(ac, card_emb, ce_w1, ce_b1, ce_w2, ce_b2,
                 as_w1, as_b1, as_b3):
    """Per-action biases + bonus matrix from action_card_indices [30, 4]."""
    ac = np.asarray(ac, np.int64)
    mask = ac != 0
    combo_size = mask.sum(1).astype(np.float32)
    values = np.where(mask, (ac - 1) % 13 + 1, 0)
    has_valid = mask.any(1)
    fidx = np.argmax(mask, axis=1)
    fv = values[np.arange(ac.shape[0]), fidx]
    same = np.where(mask, values == fv[:, None], True).all(1).astype(np.float32)
    vf = values.astype(np.float32)
    attack = np.where(values == 1, 1.0,
             np.where(values == 11, 10.0,
             np.where(values == 12, 15.0,
             np.where(values == 13, 20.0, vf))))
    total = (attack * mask).sum(1).astype(np.float32)
    suits = np.where(mask, (ac - 1) // 13 + 1, 0)
    uniq = sum((suits == s).any(1) for s in (1, 2, 3, 4)).astype(np.float32)
    ace = ((values == 1) & mask).any(1).astype(np.float32)
    valid = ((combo_size <= 4.0) & ((same > 0) | (ace > 0))).astype(np.float32)
    feats = np.stack([combo_size, same, total, uniq, ace, valid], 1)
    feats = np.where(has_valid[:, None], feats, 0.0).astype(np.float32)

    emb = card_emb[ac]                                   # [30, 4, 32]
    m = mask.astype(np.float32)[..., None]
    cnt = np.maximum(m.sum(1), 1.0)
    act_emb = (emb * m).sum(1) / cnt
    act_emb = np.where(has_valid[:, None], act_emb, 0.0).astype(np.float32)
    combo_enc = np.maximum(feats @ ce_w1 + ce_b1, 0.0) @ ce_w2 + ce_b2

    action_bias = act_emb @ as_w1[HID:HID + E] + combo_enc @ as_w1[HID + E:] + as_b1

    strength = feats[:, 2] / 20.0
    b3 = float(as_b3[0])
    Bm1 = np.zeros((4, AP_ + 2), np.float32)
    for a in range(A):
        if has_valid[a]:
            col = np.array([strength[a], 1.0 - strength[a], 0.0, 0.0])
        else:
            col = np.array([0.0, 0.0, 0.0, 2.0])
        Bm1[:, a] = col + b3
    Bm1[:, AP_] = 1.0                                    # denominator column
    ab = np.zeros((AP_, 64), np.float32)
    ab[:A] = action_bias
    return ab, Bm1


def _prep(inputs):
    """Fold weights, build per-core input maps. Returns (in_maps, consts)."""
    f32 = lambda x: np.ascontiguousarray(np.asarray(x), dtype=np.float32)
    hc = np.asarray(inputs["hand_cards"])        # [B, 8] int
    ec = np.asarray(inputs["enemy_card"])        # [B]
    hs = np.asarray(inputs["hand_size"])         # [B]
    gs = f32(inputs["game_state"])               # [B, 10]
    dc = f32(inputs["discard_pile_cards"])       # [B, 54]

    card_emb = _card_table(f32(inputs["val_emb"]), f32(inputs["suit_emb"]),
                           f32(inputs["type_emb"]))
    card_emb1 = np.concatenate([card_emb, np.ones((54, 1), np.float32)], 1)

    he_wv, he_bv = f32(inputs["he_wv"]), f32(inputs["he_bv"])
    he_wo, he_bo = f32(inputs["he_wo"]), f32(inputs["he_bo"])
    Mc = np.concatenate([he_wv @ he_wo, (he_bv @ he_wo + he_bo)[None]], 0)  # [33,32]
    A0s = 8.0 * (card_emb1 @ Mc)                                   # [54, 32]

    cx_w1, cx_b1 = f32(inputs["cx_w1"]), f32(inputs["cx_b1"])
    W1h = np.ascontiguousarray(cx_w1[0:E])                         # [32, 128]
    A2 = card_emb @ cx_w1[E:2 * E]                                 # [54, 128]
    W1s = cx_w1[2 * E:2 * E + 32]                                  # [32, 128]
    W1d = np.ascontiguousarray(cx_w1[2 * E + 32:])                 # [54, 128]
    se_w1, se_b1 = f32(inputs["se_w1"]).copy(), f32(inputs["se_b1"])
    se_w2, se_b2 = f32(inputs["se_w2"]), f32(inputs["se_b2"])
    U = se_w2 @ W1s                                                # [64, 128]
    b1f = cx_b1 + se_b2 @ W1s                                      # [128]
    se_w1[19] /= 4.0          # device computes suit-diversity count 0..4

    cxw2, cxb2 = f32(inputs["cx_w2"]), f32(inputs["cx_b2"])
    cxw3, cxb3 = f32(inputs["cx_w3"]), f32(inputs["cx_b3"])
    atw1, atb1 = f32(inputs["atc_w1"]), f32(inputs["atc_b1"])

    as_w1, as_b1 = f32(inputs["as_w1"]), f32(inputs["as_b1"])
    as_w2, as_b2 = f32(inputs["as_w2"]), f32(inputs["as_b2"])
    as_w3, as_b3 = f32(inputs["as_w3"]), f32(inputs["as_b3"])
    ab, Bm1 = _action_fold(inputs["action_card_indices"], card_emb,
                           f32(inputs["ce_w1"]), f32(inputs["ce_b1"]),
                           f32(inputs["ce_w2"]), f32(inputs["ce_b2"]),
                           as_w1, as_b1, as_b3)
    W1c = as_w1[:HID]                                              # [128, 64]
    # fold cx_w3 through the action scorer & type classifier
    Wfold = cxw3 @ W1c                                             # [128, 64]
    Wfoldd = np.concatenate([Wfold, Wfold], 1)                     # [128, 128]
    bias64 = cxb3 @ W1c                                            # [64]
    ab = ab + bias64[None, :]                                      # [32, 64]
    atw1p = cxw3 @ atw1                                            # [128, 64]
    atb1p = atb1 + cxb3 @ atw1                                     # [64]

    abp = np.zeros((128, 16), np.float32)
    for p in range(16):
        abp[0:64, p] = ab[2 * p]
        abp[64:128, p] = ab[2 * p + 1]
    W2blk = np.zeros((128, 64), np.float32)
    W2blk[0:64, 0:32] = as_w2
    W2blk[64:128, 32:64] = as_w2
    b2q = np.tile(as_b2, 4).astype(np.float32)                     # [128]
    w3blk = np.zeros((128, 4), np.float32)
    for i in range(4):
        w3blk[32 * i:32 * i + 32, i] = as_w3[:, 0]

    # sew1 padded to sh_in layout: rows 0:10 strat-w, 10:32 zero, 32:42 gs-w
    sew1v2 = np.zeros((42, 64), np.float32)
    sew1v2[0:10] = se_w1[10:20]
    sew1v2[32:42] = se_w1[0:10]

    # ---- fp16 const blob: each const occupies [0:P, c0:c0+W]
    M1 = np.zeros((128, 128), np.float32)
    M1[0:64] = U
    M1[64:118] = A2
    M2 = np.zeros((128, 128), np.float32)
    M2[0:32] = W1h
    M2[64:118] = W1d
    f16consts = [
        ("ident", np.eye(128, dtype=np.float32), 0),
        ("sew1", sew1v2, 0), ("A0s", A0s, 64), ("M1", M1, 0), ("M2", M2, 0),
        ("cxw2", cxw2, 0), ("Wfoldd", Wfoldd, 0),
        ("atw1p", atw1p, 0), ("atw2", f32(inputs["atc_w2"]), 0),
        ("W2blk", W2blk, 0), ("w3blk", w3blk, 0), ("Bm1", Bm1, 0),
    ]
    cols16 = {}
    c0 = 0
    for name, arr, r0 in f16consts:
        cols16[name] = (r0, arr.shape[0], c0, arr.shape[1])
        c0 += arr.shape[1]
    blob16 = np.zeros((128, c0), BF16)
    for name, arr, r0 in f16consts:
        r0, p, s, w = cols16[name]
        blob16[r0:r0 + p, s:s + w] = arr.astype(BF16)

    # ---- fp32 bias blob (per-partition column vectors for ACT bias)
    f32consts = [("seb1", se_b1), ("b1f", b1f), ("cxb2", cxb2),
                 ("atb1p", atb1p), ("atb2", f32(inputs["atc_b2"])),
                 ("b2q", b2q), ("iota54", np.arange(64, dtype=np.float32)),
                 ("abp", abp)]
    cols32 = {}
    c0 = 0
    for name, arr in f32consts:
        a2 = np.asarray(arr, np.float32)
        if a2.ndim == 1:
            a2 = a2[:, None]
        cols32[name] = (a2.shape[0], c0, a2.shape[1])
        c0 += a2.shape[1]
    blob32 = np.zeros((128, c0), np.float32)
    for name, arr in f32consts:
        a2 = np.asarray(arr, np.float32)
        if a2.ndim == 1:
            a2 = a2[:, None]
        p, s, w = cols32[name]
        blob32[0:p, s:s + w] = a2

    consts = {"blob16": blob16, "blob32": blob32,
              "_cols16": cols16, "_cols32": cols32}

    in_maps = []
    for i in range(NCORES):
        sl = slice(i * BC, (i + 1) * BC)
        m = {"blob16": blob16, "blob32": blob32}
        inB = np.zeros((191, BC), BF16)
        inB[32:86] = dc[sl].T
        inB[117:127] = gs[sl].T
        inB[127:191] = ec[sl].astype(BF16)[None, :]
        m["inB"] = inB
        hci = hc[sl].astype(BF16)
        hcB = np.zeros((128, 144), BF16)
        hcB[:, 0:128] = hci.reshape(16, 128, 8).transpose(1, 0, 2).reshape(128, 128)
        hcB[:, 128:144] = hs[sl].astype(BF16).reshape(16, 128).T
        m["hcB"] = hcB
        in_maps.append(m)
    return in_maps, consts


# ---------------------------------------------------------------------------
# device program
# ---------------------------------------------------------------------------

def _build(consts, n_r2_sc=8, n_s1_sc=0):
    import concourse.bass as bass
    import concourse.tile as tile
    import concourse.mybir as mybir
    from concourse import bacc

    dt = mybir.dt.float32
    dth = mybir.dt.float16
    AF = mybir.ActivationFunctionType
    OP = mybir.AluOpType
    AX = mybir.AxisListType

    cols16 = consts["_cols16"]
    cols32 = consts["_cols32"]

    nc = bacc.Bacc("TRN2", target_bir_lowering=False, debug=False,
                   enable_asserts=False, num_devices=NCORES)

    din = {}
    din["blob16"] = nc.dram_tensor("blob16", list(consts["blob16"].shape), dth,
                                   kind="ExternalInput").ap()
    din["blob32"] = nc.dram_tensor("blob32", list(consts["blob32"].shape), dt,
                                   kind="ExternalInput").ap()
    din["inB"] = nc.dram_tensor("inB", [191, BC], dth, kind="ExternalInput").ap()
    din["hcB"] = nc.dram_tensor("hcB", [128, 144], dth,
                                kind="ExternalInput").ap()
    out_d = nc.dram_tensor("out", [BC, A], dt, kind="ExternalOutput").ap()
    # out rows b = 512*n + 128*s + p  ->  [n][p, s, a]
    out_r = out_d.rearrange("(n s p) a -> n p s a", n=NCH, s=4, p=128)

    with tile.TileContext(nc) as tc, ExitStack() as ctx:
        ctx.enter_context(nc.allow_low_precision(
            reason="fp16 holds small exact integers / 2e-2 tolerance"))
        cpool = ctx.enter_context(tc.tile_pool(name="consts", bufs=1))
        core = ctx.enter_context(tc.tile_pool(name="core", bufs=1))
        work = ctx.enter_context(tc.tile_pool(name="work", bufs=3))
        s1p = ctx.enter_context(tc.tile_pool(name="s1p", bufs=8))
        s2p = ctx.enter_context(tc.tile_pool(name="s2p", bufs=4))
        fout = ctx.enter_context(tc.tile_pool(name="fout", bufs=2))
        ps_f = ctx.enter_context(tc.tile_pool(name="ps_f", bufs=2, space="PSUM"))
        ps_c = ctx.enter_context(tc.tile_pool(name="ps_c", bufs=1, space="PSUM"))
        ps_z = ctx.enter_context(tc.tile_pool(name="ps_z", bufs=3, space="PSUM"))
        ps_o = ctx.enter_context(tc.tile_pool(name="ps_o", bufs=2, space="PSUM"))

        # ---- consolidated DMAs (few descriptors, spread across idle queues)
        hcB = core.tile([128, 144], dth, tag="hcB")
        nc.sync.dma_start(hcB[:], din["hcB"])
        b16 = cpool.tile(list(consts["blob16"].shape), dth, tag="b16")
        nc.sync.dma_start(b16[:], din["blob16"])
        b32 = cpool.tile(list(consts["blob32"].shape), dt, tag="b32")
        nc.sync.dma_start(b32[:], din["blob32"])

        # HAM warmup: ~5us of back-to-back dummy matmuls while waiting for
        # input DMAs + hand-feature block; flips the PE clock gate to 8/8
        # before real matmuls start (it then stays warm: no gap > 3.4us).
        wtile = ps_f.tile([128, N], dt, tag="fe", name="warmup")
        for wi in range(12):
            nc.tensor.matmul(wtile[:], b16[:, 0:128], b16[:, 0:512],
                             start=True, stop=True)

        def pe_filler(k=2):
            # dep-free matmuls: issue instantly when the PE would otherwise
            # stall, keeping the HAM activity window busy (clock stays 8/8)
            for _ in range(k):
                nc.tensor.matmul(wtile[:, 0:256], b16[:, 0:128],
                                 b16[:, 0:256], start=True, stop=True)

        def pf(k=2):
            for _ in range(k):
                nc.tensor.matmul(wtile[:, 0:128], b16[:, 0:128],
                                 b16[:, 0:128], start=True, stop=True)
        dcX = core.tile([128, BC], dth, tag="dcX")
        nc.scalar.dma_start(dcX[32:128, :], din["inB"][0:96, :])
        sh_in = core.tile([42, BC], dth, tag="sh_in")   # strat+r | zeros | gs
        nc.sync.dma_start(sh_in[11:42, :], din["inB"][96:127, :])
        ecbc = core.tile([64, BC], dth, tag="ecbc")
        nc.scalar.dma_start(ecbc[:], din["inB"][127:191, :])

        def c16(name):
            r0, p, s, w = cols16[name]
            return b16[r0:r0 + p, s:s + w]

        def c32(name):
            p, s, w = cols32[name]
            return b32[0:p, s:s + w]

        hcS = hcB[:, 0:128]
        S = core.tile([128, 176], dth, tag="S")          # 11 blocks of 16
        expl = core.tile([4, BC], dth, tag="expl")

        g = nc.vector
        v = nc.vector
        sc = nc.scalar

        iota_col = c32("iota54")
        g.tensor_copy(S[:, 0:16], hcB[:, 128:144])       # hand_size batch-major

        # ---- per-card features (DVE, batch-major fp16, exact integer ops)
        ft = {k: core.tile([128, 128], dth, tag=f"ft_{k}", name=f"ft_{k}") for k in
              ("t", "g13", "g26", "g39", "s0", "m13", "v0", "mask",
               "s0p", "ace", "face", "lowd", "low", "su1", "su2", "su3", "su4")}
        g.tensor_scalar(ft["t"][:], hcS, -1.0, None, OP.add)
        g.tensor_scalar(ft["g13"][:], ft["t"][:], 13.0, None, OP.is_ge)
        g.tensor_scalar(ft["g26"][:], ft["t"][:], 26.0, None, OP.is_ge)
        g.tensor_scalar(ft["g39"][:], ft["t"][:], 39.0, None, OP.is_ge)
        g.tensor_tensor(ft["s0"][:], ft["g13"][:], ft["g26"][:], OP.add)
        g.tensor_tensor(ft["s0"][:], ft["s0"][:], ft["g39"][:], OP.add)
        g.tensor_scalar(ft["m13"][:], ft["s0"][:], 13.0, None, OP.mult)
        g.tensor_tensor(ft["v0"][:], ft["t"][:], ft["m13"][:], OP.subtract)
        g.tensor_scalar(ft["mask"][:], hcS, 0.5, None, OP.is_ge)
        g.tensor_scalar(ft["s0p"][:], ft["s0"][:], 1.0, None, OP.add)
        g.tensor_tensor(ft["s0p"][:], ft["s0p"][:], ft["mask"][:], OP.mult)
        g.tensor_scalar(ft["ace"][:], ft["v0"][:], 0.0, None, OP.is_equal)
        g.tensor_scalar(ft["face"][:], ft["v0"][:], 10.0, None, OP.is_ge)
        g.tensor_scalar(ft["lowd"][:], ft["v0"][:], 1.0, None, OP.is_ge)
        g.tensor_scalar(ft["low"][:], ft["v0"][:], 5.0, None, OP.is_le)
        g.tensor_tensor(ft["low"][:], ft["low"][:], ft["lowd"][:], OP.mult)
        for k, s in (("su1", 1.0), ("su2", 2.0), ("su3", 3.0), ("su4", 4.0)):
            g.tensor_scalar(ft[k][:], ft["s0p"][:], s, None, OP.is_equal)

        # ---- reduce 8 cards -> per-batch sums into S blocks (DVE)
        for blk, k in ((1, "ace"), (2, "face"), (3, "low"),
                       (4, "su1"), (5, "su2"), (6, "su3"), (7, "su4")):
            src = ft[k].rearrange("p (j c) -> p j c", c=8)
            v.tensor_reduce(S[:, 16 * blk:16 * blk + 16], src, AX.X, OP.add)

        # ---- hvr, sdiv, r (batch-major small tiles)
        hsr = core.tile([128, 16], dt, tag="hsr")
        v.tensor_scalar(hsr[:], S[:, 0:16], 1e-8, None, OP.add)
        v.reciprocal(hsr[:], hsr[:])
        v.tensor_tensor(S[:, 128:144], S[:, 32:48], hsr[:], OP.mult)  # hvr
        ge = [core.tile([128, 16], dth, tag=f"ge{k}", name=f"ge{k}") for k in range(4)]
        for k in range(4):
            v.tensor_scalar(ge[k][:], S[:, 64 + 16 * k:80 + 16 * k], 0.5, None,
                            OP.is_ge)
        v.tensor_tensor(ge[0][:], ge[0][:], ge[1][:], OP.add)
        v.tensor_tensor(ge[2][:], ge[2][:], ge[3][:], OP.add)
        v.tensor_tensor(S[:, 144:160], ge[0][:], ge[2][:], OP.add)   # sdiv cnt
        rmax = core.tile([128, 16], dt, tag="rmax")
        v.tensor_scalar(rmax[:], S[:, 0:16], 1.0, None, OP.max)
        rr32 = core.tile([128, 16], dt, tag="rr32")
        v.reciprocal(rr32[:], rmax[:])
        v.tensor_copy(S[:, 160:176], rr32[:])                        # r (fp16)


        def front_gen(n, st):
            cols = slice(N * n, N * (n + 1))

            # -- rotate per-batch scalars into rows: 4 transposes of [128, 11]
            scalT = ps_f.tile([128, N], dth, tag="fe", name="scalT")
            S_kj = S.rearrange("p (k j) -> p j k", j=16)
            for s in range(4):
                nc.tensor.transpose(scalT[0:11, 128 * s:128 * (s + 1)],
                                    S_kj[:, 4 * n + s, :], c16("ident"))
            sc.activation(sh_in[0:11, cols], scalT[0:11, :], AF.Copy)

            yield

            # -- X1: rows 0:64 strat-hidden relu, 64:128 enemy one-hot
            X1 = work.tile([128, N], dth, tag="X1")
            v.tensor_scalar(X1[64:128, :], ecbc[:, cols], iota_col, None,
                            OP.is_equal)
            shp = ps_f.tile([128, N], dt, tag="fe")
            nc.tensor.matmul(shp[0:64, :], c16("sew1"), sh_in[0:42, cols],
                             start=True, stop=True)
            sc.activation(X1[0:64, :], shp[0:64, :], AF.Relu, bias=c32("seb1"))

            yield

            # -- hand_ctx = (A0s^T oh) * r -> dcX rows 0:32
            rrow = work.tile([1, N], dth, tag="rrow")
            nc.sync.dma_start(rrow[:], sh_in[10:11, cols])
            r32 = work.tile([32, N], dth, tag="r32")
            nc.gpsimd.partition_broadcast(r32[:], rrow[:], channels=32)
            yps = ps_f.tile([128, N], dt, tag="fe")
            nc.tensor.matmul(yps[0:32, :], c16("A0s"), X1[64:118, :],
                             start=True, stop=True)
            v.tensor_tensor(dcX[0:32, cols], yps[0:32, :], r32[:], OP.mult)

            yield

            # -- z1 = M1^T X1 + M2^T dcX + b1f   (two full-K matmuls)
            z1 = ps_f.tile([128, N], dt, tag="fe")
            nc.tensor.matmul(z1[:], c16("M1"), X1[:], start=True, stop=False)
            nc.tensor.matmul(z1[:], c16("M2"), dcX[:, cols], start=False,
                             stop=True)
            h1 = work.tile([128, N], dth, tag="h1")
            sc.activation(h1[:], z1[:], AF.Relu, bias=c32("b1f"))

            yield

            h2p = ps_f.tile([128, N], dt, tag="fe")
            nc.tensor.matmul(h2p[:], c16("cxw2"), h1[:], start=True, stop=True)
            h2 = work.tile([128, N], dth, tag="h2")
            sc.activation(h2[:], h2p[:], AF.Relu, bias=c32("cxb2"))
            st["h2"] = h2

            yield

            # -- action-type probs (unnormalized exp), cx_w3 folded in
            tphp = ps_f.tile([128, N], dt, tag="fe")
            nc.tensor.matmul(tphp[0:64, :], c16("atw1p"), h2[:],
                             start=True, stop=True)
            tph = work.tile([64, N], dth, tag="tph")
            sc.activation(tph[:], tphp[0:64, :], AF.Relu, bias=c32("atb1p"))

            yield

            tlp = ps_f.tile([128, N], dt, tag="fe")
            nc.tensor.matmul(tlp[0:4, :], c16("atw2"), tph[:],
                             start=True, stop=True)
            sc.activation(expl[:, cols], tlp[0:4, :], AF.Exp, bias=c32("atb2"))

            yield

            # -- action MLP input: ctx1 duplicated pair (cx_w3 folded)
            ctx1d = ps_c.tile([128, N], dt, tag="ctx1d")
            nc.tensor.matmul(ctx1d[:], c16("Wfoldd"), h2[:], start=True,
                             stop=True)
            c1d = work.tile([128, N], dth, tag="c1d")
            sc.activation(c1d[:], ctx1d[:], AF.Copy)
            st["c1d"] = c1d

        def action_gen(n, st):
            c1d = st["c1d"]
            score = ps_o.tile([128, 128], dt, tag="fin")
            abp = c32("abp")
            for q in range(8):
                s1pair = []
                for p in (2 * q, 2 * q + 1):
                    t1 = s1p.tile([128, N], dth, tag="s1",
                                  name=f"s1_{n}_{p}", bufs=6)
                    v.tensor_scalar(t1[:], c1d[:], abp[:, p:p + 1], 0.0,
                                    OP.add, OP.max)
                    s1pair.append(t1)
                z2q = ps_z.tile([128, N], dt, tag="z2", name=f"z2_{n}_{q}")
                nc.tensor.matmul(z2q[0:64, :], c16("W2blk"), s1pair[0][:],
                                 start=True, stop=True)
                nc.tensor.matmul(z2q[64:128, :], c16("W2blk"), s1pair[1][:],
                                 start=True, stop=True)
                on_sc = q < n_r2_sc
                t = s2p.tile([128, N], dth, tag="s2a" if on_sc else "s2v",
                             name=f"s2_{n}_{q}", bufs=4)
                if on_sc:
                    sc.activation(t[:], z2q[:], AF.Relu, bias=c32("b2q"))
                else:
                    v.tensor_scalar(t[:], z2q[:], c32("b2q"), 0.0,
                                    OP.add, OP.max)
                for s in range(4):
                    nc.tensor.matmul(score[:, 32 * s + 4 * q:32 * s + 4 * q + 4],
                                     t[:, 128 * s:128 * (s + 1)],
                                     c16("w3blk"), start=True, stop=True)
                yield

            numer = ps_o.tile([128, 136], dt, tag="fin")
            for s in range(4):
                nc.tensor.matmul(numer[:, 34 * s:34 * (s + 1)],
                                 expl[:, N * n + 128 * s:N * n + 128 * (s + 1)],
                                 c16("Bm1"), start=True, stop=True)
            yield

            recipT = fout.tile([128, 4], dt, tag="recip")
            den = numer.rearrange("p (s c) -> p s c", c=34)[:, :, 32]
            v.reciprocal(recipT[:], den)
            tmp = fout.tile([128, 120], dt, tag="tmp")
            for s in range(4):
                v.tensor_scalar(tmp[:, 30 * s:30 * (s + 1)],
                                numer[:, 34 * s:34 * s + 30],
                                recipT[:, s:s + 1], None, OP.mult)
            sc_ap = score.rearrange("p (s c) -> p s c", c=32)[:, :, 0:30]
            outT = fout.tile([128, 120], dt, tag="outT")
            v.tensor_tensor(outT.rearrange("p (s c) -> p s c", c=30),
                            tmp.rearrange("p (s c) -> p s c", c=30),
                            sc_ap, OP.add)
            nc.sync.dma_start(out_r[n],
                              outT.rearrange("p (s c) -> p s c", c=30))

        def drain(g):
            for _ in g:
                pass

        # software pipeline: chunk m's action stage interleaves with
        # chunk (m+2)'s front stage so the in-order PE always has work
        sts = [dict() for _ in range(NCH)]
        fgens = [front_gen(n, sts[n]) for n in range(NCH)]
        agens = [action_gen(n, sts[n]) for n in range(NCH)]
        drain(fgens[0])
        drain(fgens[1])
        for m in range(NCH):
            f = fgens[m + 2] if m + 2 < NCH else None
            a = agens[m]
            alive = True
            while alive:
                alive = False
                try:
                    next(a)
                    alive = True
                except StopIteration:
                    pass
                if f is not None:
                    try:
                        next(f)
                        alive = True
                    except StopIteration:
                        pass

    nc.compile()
    return nc


def _get_program(consts):
    key = "prog"
    if key not in _cache:
        _cache[key] = _build(consts)
    return _cache[key]


def kernel(**inputs):
    in_maps, consts = _prep(inputs)
    nc = _get_program(consts)
    from concourse.bass_utils import run_bass_kernel_spmd
    res = run_bass_kernel_spmd(nc, in_maps, core_ids=list(range(NCORES)))
    out = np.concatenate([res.results[i]["out"] for i in range(NCORES)], 0)
    return out.astype(np.float32)


# revision 37
# speedup vs baseline: 1.2538x; 1.0032x over previous
"""Trainium2 Bass kernel for nn_EnhancedCardAwarePolicy.

Strategy: pure data-parallel across 8 NeuronCores (batch 16384 -> 2048/core).

Key algebraic simplifications (exactly value-preserving vs the reference):
  * The hand self-attention MHA is dead code: the cross-attention that
    consumes it has sequence length 1, so its softmax is identically 1 and
    its output is independent of the query.  hand_ctx reduces to
        (8 / max(hand_size,1)) * (enemy_emb @ he_wv @ he_wo + he_bv @ he_wo + he_bo)
  * Card encodings are pure functions of the card index 0..53 -> fold the
    embedding tables into one [54, 32] table, and fold that table through
    the downstream linear layers, so the enemy-card path becomes matmuls
    against a one-hot [54, B] matrix built on-device.
  * strat_ctx's second linear layer is folded into cx_w1.
  * cx_w3 is folded into the action scorer's first layer (as_w1[:HID]) and
    the action-type classifier's first layer, so the `ctx` activation is
    never materialized.
  * The per-action tables are folded into per-action bias vectors on host.
  * softmax+bonus is computed unnormalized: out = score + (expl@Bm)/(expl@1).

Device layout: feature-major [D, B] activations in fp16 (fp32 PSUM accum);
relu(ctx1 + per-action-bias) runs as DVE tensor_scalar ops from SBUF at 4x
packed rate; z2 action matmuls col-tile the PE array in concurrent pairs.
"""

import numpy as np
import ml_dtypes
from contextlib import ExitStack

BF16 = np.float16

B = 16384
NCORES = 8
BC = B // NCORES          # 2048 batch rows per core
NCH = 4                   # chunks per core
N = BC // NCH             # 512 batch columns per chunk
A = 30                    # real actions
AP_ = 32                  # padded actions
E = 32
HID = 128

_cache = {}


# ---------------------------------------------------------------------------
# host-side folding
# ---------------------------------------------------------------------------

def _card_table(val_emb, suit_emb, type_emb):
    """[54, 32] full card encoding table, matching _encode_cards."""
    c = np.arange(54)
    invalid = (c == 0) | (c == 53)
    v = np.where(invalid, 0, (c - 1) % 13 + 1)
    s = np.where(invalid, 0, (c - 1) // 13 + 1)
    ce = np.concatenate([val_emb[v], suit_emb[s]], axis=-1)          # [54, 32]
    ct = np.where(v == 11, 1, np.where(v == 12, 2, np.where(v == 13, 3, 0)))
    te = type_emb[ct]                                                # [54, 8]
    pad = np.zeros((54, E - te.shape[-1]), np.float32)
    return (ce + np.concatenate([te, pad], axis=-1)).astype(np.float32)


def _action_fold(ac, card_emb, ce_w1, ce_b1, ce_w2, ce_b2,
                 as_w1, as_b1, as_b3):
    """Per-action biases + bonus matrix from action_card_indices [30, 4]."""
    ac = np.asarray(ac, np.int64)
    mask = ac != 0
    combo_size = mask.sum(1).astype(np.float32)
    values = np.where(mask, (ac - 1) % 13 + 1, 0)
    has_valid = mask.any(1)
    fidx = np.argmax(mask, axis=1)
    fv = values[np.arange(ac.shape[0]), fidx]
    same = np.where(mask, values == fv[:, None], True).all(1).astype(np.float32)
    vf = values.astype(np.float32)
    attack = np.where(values == 1, 1.0,
             np.where(values == 11, 10.0,
             np.where(values == 12, 15.0,
             np.where(values == 13, 20.0, vf))))
    total = (attack * mask).sum(1).astype(np.float32)
    suits = np.where(mask, (ac - 1) // 13 + 1, 0)
    uniq = sum((suits == s).any(1) for s in (1, 2, 3, 4)).astype(np.float32)
    ace = ((values == 1) & mask).any(1).astype(np.float32)
    valid = ((combo_size <= 4.0) & ((same > 0) | (ace > 0))).astype(np.float32)
    feats = np.stack([combo_size, same, total, uniq, ace, valid], 1)
    feats = np.where(has_valid[:, None], feats, 0.0).astype(np.float32)

    emb = card_emb[ac]                                   # [30, 4, 32]
    m = mask.astype(np.float32)[..., None]
    cnt = np.maximum(m.sum(1), 1.0)
    act_emb = (emb * m).sum(1) / cnt
    act_emb = np.where(has_valid[:, None], act_emb, 0.0).astype(np.float32)
    combo_enc = np.maximum(feats @ ce_w1 + ce_b1, 0.0) @ ce_w2 + ce_b2

    action_bias = act_emb @ as_w1[HID:HID + E] + combo_enc @ as_w1[HID + E:] + as_b1

    strength = feats[:, 2] / 20.0
    b3 = float(as_b3[0])
    Bm1 = np.zeros((4, AP_ + 2), np.float32)
    for a in range(A):
        if has_valid[a]:
            col = np.array([strength[a], 1.0 - strength[a], 0.0, 0.0])
        else:
            col = np.array([0.0, 0.0, 0.0, 2.0])
        Bm1[:, a] = col + b3
    Bm1[:, AP_] = 1.0                                    # denominator column
    ab = np.zeros((AP_, 64), np.float32)
    ab[:A] = action_bias
    return ab, Bm1


def _prep(inputs):
    """Fold weights, build per-core input maps. Returns (in_maps, consts)."""
    f32 = lambda x: np.ascontiguousarray(np.asarray(x), dtype=np.float32)
    hc = np.asarray(inputs["hand_cards"])        # [B, 8] int
    ec = np.asarray(inputs["enemy_card"])        # [B]
    hs = np.asarray(inputs["hand_size"])         # [B]
    gs = f32(inputs["game_state"])               # [B, 10]
    dc = f32(inputs["discard_pile_cards"])       # [B, 54]

    card_emb = _card_table(f32(inputs["val_emb"]), f32(inputs["suit_emb"]),
                           f32(inputs["type_emb"]))
    card_emb1 = np.concatenate([card_emb, np.ones((54, 1), np.float32)], 1)

    he_wv, he_bv = f32(inputs["he_wv"]), f32(inputs["he_bv"])
    he_wo, he_bo = f32(inputs["he_wo"]), f32(inputs["he_bo"])
    Mc = np.concatenate([he_wv @ he_wo, (he_bv @ he_wo + he_bo)[None]], 0)  # [33,32]
    A0s = 8.0 * (card_emb1 @ Mc)                                   # [54, 32]

    cx_w1, cx_b1 = f32(inputs["cx_w1"]), f32(inputs["cx_b1"])
    W1h = np.ascontiguousarray(cx_w1[0:E])                         # [32, 128]
    A2 = card_emb @ cx_w1[E:2 * E]                                 # [54, 128]
    W1s = cx_w1[2 * E:2 * E + 32]                                  # [32, 128]
    W1d = np.ascontiguousarray(cx_w1[2 * E + 32:])                 # [54, 128]
    se_w1, se_b1 = f32(inputs["se_w1"]).copy(), f32(inputs["se_b1"])
    se_w2, se_b2 = f32(inputs["se_w2"]), f32(inputs["se_b2"])
    U = se_w2 @ W1s                                                # [64, 128]
    b1f = cx_b1 + se_b2 @ W1s                                      # [128]
    se_w1[19] /= 4.0          # device computes suit-diversity count 0..4

    cxw2, cxb2 = f32(inputs["cx_w2"]), f32(inputs["cx_b2"])
    cxw3, cxb3 = f32(inputs["cx_w3"]), f32(inputs["cx_b3"])
    atw1, atb1 = f32(inputs["atc_w1"]), f32(inputs["atc_b1"])

    as_w1, as_b1 = f32(inputs["as_w1"]), f32(inputs["as_b1"])
    as_w2, as_b2 = f32(inputs["as_w2"]), f32(inputs["as_b2"])
    as_w3, as_b3 = f32(inputs["as_w3"]), f32(inputs["as_b3"])
    ab, Bm1 = _action_fold(inputs["action_card_indices"], card_emb,
                           f32(inputs["ce_w1"]), f32(inputs["ce_b1"]),
                           f32(inputs["ce_w2"]), f32(inputs["ce_b2"]),
                           as_w1, as_b1, as_b3)
    W1c = as_w1[:HID]                                              # [128, 64]
    # fold cx_w3 through the action scorer & type classifier
    Wfold = cxw3 @ W1c                                             # [128, 64]
    Wfoldd = np.concatenate([Wfold, Wfold], 1)                     # [128, 128]
    bias64 = cxb3 @ W1c                                            # [64]
    ab = ab + bias64[None, :]                                      # [32, 64]
    atw1p = cxw3 @ atw1                                            # [128, 64]
    atb1p = atb1 + cxb3 @ atw1                                     # [64]

    abp = np.zeros((128, 16), np.float32)
    for p in range(16):
        abp[0:64, p] = ab[2 * p]
        abp[64:128, p] = ab[2 * p + 1]
    W2blk = np.zeros((128, 64), np.float32)
    W2blk[0:64, 0:32] = as_w2
    W2blk[64:128, 32:64] = as_w2
    b2q = np.tile(as_b2, 4).astype(np.float32)                     # [128]
    w3blk = np.zeros((128, 4), np.float32)
    for i in range(4):
        w3blk[32 * i:32 * i + 32, i] = as_w3[:, 0]

    # sew1 padded to sh_in layout: rows 0:10 strat-w, 10:32 zero, 32:42 gs-w
    sew1v2 = np.zeros((42, 64), np.float32)
    sew1v2[0:10] = se_w1[10:20]
    sew1v2[32:42] = se_w1[0:10]

    # ---- fp16 const blob: each const occupies [0:P, c0:c0+W]
    M1 = np.zeros((128, 128), np.float32)
    M1[0:64] = U
    M1[64:118] = A2
    M2 = np.zeros((128, 128), np.float32)
    M2[0:32] = W1h
    M2[64:118] = W1d
    f16consts = [
        ("ident", np.eye(128, dtype=np.float32), 0),
        ("sew1", sew1v2, 0), ("A0s", A0s, 64), ("M1", M1, 0), ("M2", M2, 0),
        ("cxw2", cxw2, 0), ("Wfoldd", Wfoldd, 0),
        ("atw1p", atw1p, 0), ("atw2", f32(inputs["atc_w2"]), 0),
        ("W2blk", W2blk, 0), ("w3blk", w3blk, 0), ("Bm1", Bm1, 0),
    ]
    cols16 = {}
    c0 = 0
    for name, arr, r0 in f16consts:
        cols16[name] = (r0, arr.shape[0], c0, arr.shape[1])
        c0 += arr.shape[1]
    blob16 = np.zeros((128, c0), BF16)
    for name, arr, r0 in f16consts:
        r0, p, s, w = cols16[name]
        blob16[r0:r0 + p, s:s + w] = arr.astype(BF16)

    # ---- fp32 bias blob (per-partition column vectors for ACT bias)
    f32consts = [("seb1", se_b1), ("b1f", b1f), ("cxb2", cxb2),
                 ("atb1p", atb1p), ("atb2", f32(inputs["atc_b2"])),
                 ("b2q", b2q), ("iota54", np.arange(64, dtype=np.float32)),
                 ("abp", abp)]
    cols32 = {}
    c0 = 0
    for name, arr in f32consts:
        a2 = np.asarray(arr, np.float32)
        if a2.ndim == 1:
            a2 = a2[:, None]
        cols32[name] = (a2.shape[0], c0, a2.shape[1])
        c0 += a2.shape[1]
    blob32 = np.zeros((128, c0), np.float32)
    for name, arr in f32consts:
        a2 = np.asarray(arr, np.float32)
        if a2.ndim == 1:
            a2 = a2[:, None]
        p, s, w = cols32[name]
        blob32[0:p, s:s + w] = a2

    consts = {"blob16": blob16, "blob32": blob32,
              "_cols16": cols16, "_cols32": cols32}

    in_maps = []
    for i in range(NCORES):
        sl = slice(i * BC, (i + 1) * BC)
        m = {"blob16": blob16, "blob32": blob32}
        inB = np.zeros((191, BC), BF16)
        inB[32:86] = dc[sl].T
        inB[117:127] = gs[sl].T
        inB[127:191] = ec[sl].astype(BF16)[None, :]
        m["inB"] = inB
        hci = hc[sl].astype(BF16)
        hcB = np.zeros((128, 144), BF16)
        hcB[:, 0:128] = hci.reshape(16, 128, 8).transpose(1, 0, 2).reshape(128, 128)
        hcB[:, 128:144] = hs[sl].astype(BF16).reshape(16, 128).T
        m["hcB"] = hcB
        in_maps.append(m)
    return in_maps, consts


# ---------------------------------------------------------------------------
# device program
# ---------------------------------------------------------------------------

def _build(consts, n_r2_sc=7, n_s1_sc=0):
    import concourse.bass as bass
    import concourse.tile as tile
    import concourse.mybir as mybir
    from concourse import bacc

    dt = mybir.dt.float32
    dth = mybir.dt.float16
    AF = mybir.ActivationFunctionType
    OP = mybir.AluOpType
    AX = mybir.AxisListType

    cols16 = consts["_cols16"]
    cols32 = consts["_cols32"]

    nc = bacc.Bacc("TRN2", target_bir_lowering=False, debug=False,
                   enable_asserts=False, num_devices=NCORES)

    din = {}
    din["blob16"] = nc.dram_tensor("blob16", list(consts["blob16"].shape), dth,
                                   kind="ExternalInput").ap()
    din["blob32"] = nc.dram_tensor("blob32", list(consts["blob32"].shape), dt,
                                   kind="ExternalInput").ap()
    din["inB"] = nc.dram_tensor("inB", [191, BC], dth, kind="ExternalInput").ap()
    din["hcB"] = nc.dram_tensor("hcB", [128, 144], dth,
                                kind="ExternalInput").ap()
    out_d = nc.dram_tensor("out", [BC, A], dt, kind="ExternalOutput").ap()
    # out rows b = 512*n + 128*s + p  ->  [n][p, s, a]
    out_r = out_d.rearrange("(n s p) a -> n p s a", n=NCH, s=4, p=128)

    with tile.TileContext(nc) as tc, ExitStack() as ctx:
        ctx.enter_context(nc.allow_low_precision(
            reason="fp16 holds small exact integers / 2e-2 tolerance"))
        cpool = ctx.enter_context(tc.tile_pool(name="consts", bufs=1))
        core = ctx.enter_context(tc.tile_pool(name="core", bufs=1))
        work = ctx.enter_context(tc.tile_pool(name="work", bufs=4))
        s1p = ctx.enter_context(tc.tile_pool(name="s1p", bufs=8))
        s2p = ctx.enter_context(tc.tile_pool(name="s2p", bufs=4))
        fout = ctx.enter_context(tc.tile_pool(name="fout", bufs=2))
        ps_f = ctx.enter_context(tc.tile_pool(name="ps_f", bufs=2, space="PSUM"))
        ps_c = ctx.enter_context(tc.tile_pool(name="ps_c", bufs=1, space="PSUM"))
        ps_z = ctx.enter_context(tc.tile_pool(name="ps_z", bufs=3, space="PSUM"))
        ps_o = ctx.enter_context(tc.tile_pool(name="ps_o", bufs=2, space="PSUM"))

        # ---- consolidated DMAs (few descriptors, spread across idle queues)
        hcB = core.tile([128, 144], dth, tag="hcB")
        nc.sync.dma_start(hcB[:], din["hcB"])
        b16 = cpool.tile(list(consts["blob16"].shape), dth, tag="b16")
        nc.sync.dma_start(b16[:], din["blob16"])
        b32 = cpool.tile(list(consts["blob32"].shape), dt, tag="b32")
        nc.sync.dma_start(b32[:], din["blob32"])

        # HAM warmup: ~5us of back-to-back dummy matmuls while waiting for
        # input DMAs + hand-feature block; flips the PE clock gate to 8/8
        # before real matmuls start (it then stays warm: no gap > 3.4us).
        wtile = ps_f.tile([128, N], dt, tag="fe", name="warmup")
        for wi in range(12):
            nc.tensor.matmul(wtile[:], b16[:, 0:128], b16[:, 0:512],
                             start=True, stop=True)

        def pe_filler(k=2):
            # dep-free matmuls: issue instantly when the PE would otherwise
            # stall, keeping the HAM activity window busy (clock stays 8/8)
            for _ in range(k):
                nc.tensor.matmul(wtile[:, 0:256], b16[:, 0:128],
                                 b16[:, 0:256], start=True, stop=True)

        def pf(k=2):
            for _ in range(k):
                nc.tensor.matmul(wtile[:, 0:128], b16[:, 0:128],
                                 b16[:, 0:128], start=True, stop=True)
        dcX = core.tile([128, BC], dth, tag="dcX")
        nc.scalar.dma_start(dcX[32:128, :], din["inB"][0:96, :])
        sh_in = core.tile([42, BC], dth, tag="sh_in")   # strat+r | zeros | gs
        nc.sync.dma_start(sh_in[11:42, :], din["inB"][96:127, :])
        ecbc = core.tile([64, BC], dth, tag="ecbc")
        nc.scalar.dma_start(ecbc[:], din["inB"][127:191, :])

        def c16(name):
            r0, p, s, w = cols16[name]
            return b16[r0:r0 + p, s:s + w]

        def c32(name):
            p, s, w = cols32[name]
            return b32[0:p, s:s + w]

        hcS = hcB[:, 0:128]
        S = core.tile([128, 176], dth, tag="S")          # 11 blocks of 16
        expl = core.tile([4, BC], dth, tag="expl")

        g = nc.vector
        v = nc.vector
        sc = nc.scalar

        iota_col = c32("iota54")
        g.tensor_copy(S[:, 0:16], hcB[:, 128:144])       # hand_size batch-major

        # ---- per-card features (DVE, batch-major fp16, exact integer ops)
        ft = {k: core.tile([128, 128], dth, tag=f"ft_{k}", name=f"ft_{k}") for k in
              ("t", "g13", "g26", "g39", "s0", "m13", "v0", "mask",
               "s0p", "ace", "face", "lowd", "low", "su1", "su2", "su3", "su4")}
        g.tensor_scalar(ft["t"][:], hcS, -1.0, None, OP.add)
        g.tensor_scalar(ft["g13"][:], ft["t"][:], 13.0, None, OP.is_ge)
        g.tensor_scalar(ft["g26"][:], ft["t"][:], 26.0, None, OP.is_ge)
        g.tensor_scalar(ft["g39"][:], ft["t"][:], 39.0, None, OP.is_ge)
        g.tensor_tensor(ft["s0"][:], ft["g13"][:], ft["g26"][:], OP.add)
        g.tensor_tensor(ft["s0"][:], ft["s0"][:], ft["g39"][:], OP.add)
        g.tensor_scalar(ft["m13"][:], ft["s0"][:], 13.0, None, OP.mult)
        g.tensor_tensor(ft["v0"][:], ft["t"][:], ft["m13"][:], OP.subtract)
        g.tensor_scalar(ft["mask"][:], hcS, 0.5, None, OP.is_ge)
        g.tensor_scalar(ft["s0p"][:], ft["s0"][:], 1.0, None, OP.add)
        g.tensor_tensor(ft["s0p"][:], ft["s0p"][:], ft["mask"][:], OP.mult)
        g.tensor_scalar(ft["ace"][:], ft["v0"][:], 0.0, None, OP.is_equal)
        g.tensor_scalar(ft["face"][:], ft["v0"][:], 10.0, None, OP.is_ge)
        g.tensor_scalar(ft["lowd"][:], ft["v0"][:], 1.0, None, OP.is_ge)
        g.tensor_scalar(ft["low"][:], ft["v0"][:], 5.0, None, OP.is_le)
        g.tensor_tensor(ft["low"][:], ft["low"][:], ft["lowd"][:], OP.mult)
        for k, s in (("su1", 1.0), ("su2", 2.0), ("su3", 3.0), ("su4", 4.0)):
            g.tensor_scalar(ft[k][:], ft["s0p"][:], s, None, OP.is_equal)

        # ---- reduce 8 cards -> per-batch sums into S blocks (DVE)
        for blk, k in ((1, "ace"), (2, "face"), (3, "low"),
                       (4, "su1"), (5, "su2"), (6, "su3"), (7, "su4")):
            src = ft[k].rearrange("p (j c) -> p j c", c=8)
            v.tensor_reduce(S[:, 16 * blk:16 * blk + 16], src, AX.X, OP.add)

        # ---- hvr, sdiv, r (batch-major small tiles)
        hsr = core.tile([128, 16], dt, tag="hsr")
        v.tensor_scalar(hsr[:], S[:, 0:16], 1e-8, None, OP.add)
        v.reciprocal(hsr[:], hsr[:])
        v.tensor_tensor(S[:, 128:144], S[:, 32:48], hsr[:], OP.mult)  # hvr
        ge = [core.tile([128, 16], dth, tag=f"ge{k}", name=f"ge{k}") for k in range(4)]
        for k in range(4):
            v.tensor_scalar(ge[k][:], S[:, 64 + 16 * k:80 + 16 * k], 0.5, None,
                            OP.is_ge)
        v.tensor_tensor(ge[0][:], ge[0][:], ge[1][:], OP.add)
        v.tensor_tensor(ge[2][:], ge[2][:], ge[3][:], OP.add)
        v.tensor_tensor(S[:, 144:160], ge[0][:], ge[2][:], OP.add)   # sdiv cnt
        rmax = core.tile([128, 16], dt, tag="rmax")
        v.tensor_scalar(rmax[:], S[:, 0:16], 1.0, None, OP.max)
        rr32 = core.tile([128, 16], dt, tag="rr32")
        v.reciprocal(rr32[:], rmax[:])
        v.tensor_copy(S[:, 160:176], rr32[:])                        # r (fp16)


        def front_gen(n, st):
            cols = slice(N * n, N * (n + 1))

            # -- rotate per-batch scalars into rows: 4 transposes of [128, 11]
            scalT = ps_f.tile([128, N], dth, tag="fe", name="scalT")
            S_kj = S.rearrange("p (k j) -> p j k", j=16)
            for s in range(4):
                nc.tensor.transpose(scalT[0:11, 128 * s:128 * (s + 1)],
                                    S_kj[:, 4 * n + s, :], c16("ident"))
            sc.activation(sh_in[0:11, cols], scalT[0:11, :], AF.Copy)

            yield

            # -- X1: rows 0:64 strat-hidden relu, 64:128 enemy one-hot
            X1 = work.tile([128, N], dth, tag="X1")
            v.tensor_scalar(X1[64:128, :], ecbc[:, cols], iota_col, None,
                            OP.is_equal)
            shp = ps_f.tile([128, N], dt, tag="fe")
            nc.tensor.matmul(shp[0:64, :], c16("sew1"), sh_in[0:42, cols],
                             start=True, stop=True)
            sc.activation(X1[0:64, :], shp[0:64, :], AF.Relu, bias=c32("seb1"))

            yield

            # -- hand_ctx = (A0s^T oh) * r -> dcX rows 0:32
            rrow = work.tile([1, N], dth, tag="rrow")
            nc.sync.dma_start(rrow[:], sh_in[10:11, cols])
            r32 = work.tile([32, N], dth, tag="r32")
            nc.gpsimd.partition_broadcast(r32[:], rrow[:], channels=32)
            yps = ps_f.tile([128, N], dt, tag="fe")
            nc.tensor.matmul(yps[0:32, :], c16("A0s"), X1[64:118, :],
                             start=True, stop=True)
            v.tensor_tensor(dcX[0:32, cols], yps[0:32, :], r32[:], OP.mult)

            yield

            # -- z1 = M1^T X1 + M2^T dcX + b1f   (two full-K matmuls)
            z1 = ps_f.tile([128, N], dt, tag="fe")
            nc.tensor.matmul(z1[:], c16("M1"), X1[:], start=True, stop=False)
            nc.tensor.matmul(z1[:], c16("M2"), dcX[:, cols], start=False,
                             stop=True)
            h1 = work.tile([128, N], dth, tag="h1")
            sc.activation(h1[:], z1[:], AF.Relu, bias=c32("b1f"))

            yield

            h2p = ps_f.tile([128, N], dt, tag="fe")
            nc.tensor.matmul(h2p[:], c16("cxw2"), h1[:], start=True, stop=True)
            h2 = work.tile([128, N], dth, tag="h2")
            sc.activation(h2[:], h2p[:], AF.Relu, bias=c32("cxb2"))
            st["h2"] = h2

            yield

            # -- action-type probs (unnormalized exp), cx_w3 folded in
            tphp = ps_f.tile([128, N], dt, tag="fe")
            nc.tensor.matmul(tphp[0:64, :], c16("atw1p"), h2[:],
                             start=True, stop=True)
            tph = work.tile([64, N], dth, tag="tph")
            sc.activation(tph[:], tphp[0:64, :], AF.Relu, bias=c32("atb1p"))

            yield

            tlp = ps_f.tile([128, N], dt, tag="fe")
            nc.tensor.matmul(tlp[0:4, :], c16("atw2"), tph[:],
                             start=True, stop=True)
            sc.activation(expl[:, cols], tlp[0:4, :], AF.Exp, bias=c32("atb2"))

            yield

            # -- action MLP input: ctx1 duplicated pair (cx_w3 folded)
            ctx1d = ps_c.tile([128, N], dt, tag="ctx1d")
            nc.tensor.matmul(ctx1d[:], c16("Wfoldd"), h2[:], start=True,
                             stop=True)
            c1d = work.tile([128, N], dth, tag="c1d")
            sc.activation(c1d[:], ctx1d[:], AF.Copy)
            st["c1d"] = c1d

        def action_gen(n, st):
            c1d = st["c1d"]
            score = ps_o.tile([128, 128], dt, tag="fin")
            abp = c32("abp")
            for q in range(8):
                s1pair = []
                for p in (2 * q, 2 * q + 1):
                    t1 = s1p.tile([128, N], dth, tag="s1",
                                  name=f"s1_{n}_{p}", bufs=8)
                    v.tensor_scalar(t1[:], c1d[:], abp[:, p:p + 1], 0.0,
                                    OP.add, OP.max)
                    s1pair.append(t1)
                z2q = ps_z.tile([128, N], dt, tag="z2", name=f"z2_{n}_{q}")
                nc.tensor.matmul(z2q[0:64, :], c16("W2blk"), s1pair[0][:],
                                 start=True, stop=True)
                nc.tensor.matmul(z2q[64:128, :], c16("W2blk"), s1pair[1][:],
                                 start=True, stop=True)
                on_sc = q < n_r2_sc
                t = s2p.tile([128, N], dth, tag="s2a" if on_sc else "s2v",
                             name=f"s2_{n}_{q}", bufs=4)
                if on_sc:
                    sc.activation(t[:], z2q[:], AF.Relu, bias=c32("b2q"))
                else:
                    v.tensor_scalar(t[:], z2q[:], c32("b2q"), 0.0,
                                    OP.add, OP.max)
                for s in range(4):
                    nc.tensor.matmul(score[:, 32 * s + 4 * q:32 * s + 4 * q + 4],
                                     t[:, 128 * s:128 * (s + 1)],
                                     c16("w3blk"), start=True, stop=True)
                yield

            numer = ps_o.tile([128, 136], dt, tag="fin")
            for s in range(4):
                nc.tensor.matmul(numer[:, 34 * s:34 * (s + 1)],
                                 expl[:, N * n + 128 * s:N * n + 128 * (s + 1)],
                                 c16("Bm1"), start=True, stop=True)
            yield

            recipT = fout.tile([128, 4], dt, tag="recip")
            den = numer.rearrange("p (s c) -> p s c", c=34)[:, :, 32]
            v.reciprocal(recipT[:], den)
            tmp = fout.tile([128, 120], dt, tag="tmp")
            for s in range(4):
                v.tensor_scalar(tmp[:, 30 * s:30 * (s + 1)],
                                numer[:, 34 * s:34 * s + 30],
                                recipT[:, s:s + 1], None, OP.mult)
            sc_ap = score.rearrange("p (s c) -> p s c", c=32)[:, :, 0:30]
            outT = fout.tile([128, 120], dt, tag="outT")
            v.tensor_tensor(outT.rearrange("p (s c) -> p s c", c=30),
                            tmp.rearrange("p (s c) -> p s c", c=30),
                            sc_ap, OP.add)
            nc.sync.dma_start(out_r[n],
                              outT.rearrange("p (s c) -> p s c", c=30))

        def drain(g):
            for _ in g:
                pass

        # software pipeline: chunk m's action stage interleaves with
        # chunk (m+2)'s front stage so the in-order PE always has work
        sts = [dict() for _ in range(NCH)]
        fgens = [front_gen(n, sts[n]) for n in range(NCH)]
        agens = [action_gen(n, sts[n]) for n in range(NCH)]
        drain(fgens[0])
        drain(fgens[1])
        for m in range(NCH):
            f = fgens[m + 2] if m + 2 < NCH else None
            a = agens[m]
            alive = True
            while alive:
                alive = False
                try:
                    next(a)
                    alive = True
                except StopIteration:
                    pass
                if f is not None:
                    try:
                        next(f)
                        alive = True
                    except StopIteration:
                        pass

    nc.compile()
    return nc


def _get_program(consts):
    key = "prog"
    if key not in _cache:
        _cache[key] = _build(consts)
    return _cache[key]


def kernel(**inputs):
    in_maps, consts = _prep(inputs)
    nc = _get_program(consts)
    from concourse.bass_utils import run_bass_kernel_spmd
    res = run_bass_kernel_spmd(nc, in_maps, core_ids=list(range(NCORES)))
    out = np.concatenate([res.results[i]["out"] for i in range(NCORES)], 0)
    return out.astype(np.float32)
